# revision 4
# baseline (speedup 1.0000x reference)
"""Trainium2 Bass kernel for 16-head causal MHA (RMSNorm+RoPE on q,k).

Tensor-parallel over heads: 8 cores x 2 heads each. Each core computes
qkv projection for its heads, norm+rope, causal attention, and a partial
out-projection; the host sums the 8 partial outputs.

v2 layout notes (vs the original v-stationary design):
- Scores are computed transposed [k, q]; exp tiles then serve as the
  STATIONARY matmul operand for PV, with a ones-column appended to V, so
  the PV output lands as [q, v|den] in PSUM: the softmax denominator is
  column 128 and the division becomes a per-partition ACT copy-scale.
  This removes all denominator matmuls and all per-column scaling ops.
- All 128x128 transposes (phase-1 q/k blocks, phase-2 o blocks) go
  through the DMA xbar transpose engine instead of TensorE.
- RoPE is computed into a de-interleaved [odd-half | even-half] d-order,
  identically for q and k (dot products unchanged); v / out_proj keep the
  natural d-order.
- exp() is computed without max-subtraction: post-RMSNorm |q.k|/sqrt(hd)
  <= sqrt(128), so exp is bounded by ~8.2e4. Masked (upper-triangular)
  score blocks are skipped entirely; diagonal blocks get an additive -1e9.
"""
import os
import ml_dtypes
import numpy as np

import concourse.bacc as bacc
import concourse.mybir as mybir
import concourse.tile as tile
from concourse.ap import AP
from concourse.bass_utils import run_bass_kernel_spmd


def _bcast_mid(ap2d, n):
    """[128, X] -> [128, n, X] with step-0 middle dim."""
    return AP(tensor=ap2d.tensor, offset=ap2d.offset,
              ap=[list(ap2d.ap[0]), [0, n], list(ap2d.ap[1])])

F32 = mybir.dt.float32
F32R = mybir.dt.float32r
BF16 = mybir.dt.bfloat16
WDTYPE = os.environ.get("MHA_WDTYPE", "bf16")
WDT = BF16 if WDTYPE == "bf16" else F32R
AF = mybir.ActivationFunctionType
ALU = mybir.AluOpType
AX = mybir.AxisListType

N_CORES = 8
L = 2048
D = 2048
HD = 128
N_HEAD = 16
HPC = N_HEAD // N_CORES  # heads per core = 2
LT = 128                 # L-tile rows
NT = L // LT             # 16 L-tiles
HC = 128                 # hid chunk
NHC = D // HC            # 16 hid chunks
QT = 512                 # q-tile width in attention
NQT = L // QT            # 4
VW = 130                 # v row stride (128 dims + ones col + pad)
EPS = 1e-5
ROPE_BASE = 10000.0
SCALE = 1.0 / float(np.sqrt(HD))
NEG = -1.0e9


def build():
    nc = bacc.Bacc("TRN2", target_bir_lowering=False, debug=False,
                   enable_asserts=False, num_devices=N_CORES)

    # Per-core external inputs (host-prepped layouts; see prep_inputs()).
    xt = nc.dram_tensor("xt", [NT, HC, NHC, LT], WDT, kind="ExternalInput")
    wt = nc.dram_tensor("wt", [D, 6 * HD], WDT, kind="ExternalInput")
    wo = nc.dram_tensor("wo", [HD, HPC, D], WDT, kind="ExternalInput")
    w1 = nc.dram_tensor("w1", [LT, NT, HD], F32, kind="ExternalInput")
    w2 = nc.dram_tensor("w2", [LT, NT, HD], F32, kind="ExternalInput")
    mask4 = nc.dram_tensor("mask4", [128, 128], F32, kind="ExternalInput")

    out = nc.dram_tensor("out", [L, D], F32, kind="ExternalOutput")

    with tile.TileContext(nc) as tc:
        with (
            tc.tile_pool(name="const", bufs=1) as constp,
            tc.tile_pool(name="wpool", bufs=1) as wpool,
            tc.tile_pool(name="persist", bufs=1) as persist,
            tc.tile_pool(name="xin", bufs=3) as xin,
            tc.tile_pool(name="qkv", bufs=3) as qkvp,
            tc.tile_pool(name="attn", bufs=4) as attnp,
            tc.tile_pool(name="res", bufs=4) as resp,
        ):
            # ---- weights resident (w chunks first: they gate the GEMMs).
            # Split across the two HWDGE queues for startup bandwidth.
            w_sb = wpool.tile([128, NHC, 6 * HD], WDT)
            for c in range(NHC):
                eng = nc.sync if c % 2 == 0 else nc.scalar
                eng.dma_start(out=w_sb[:, c, :],
                              in_=wt[c * 128:(c + 1) * 128, :])
            w1_sb = constp.tile([128, NT, HD], F32)
            nc.gpsimd.dma_start(out=w1_sb[:, 0, :], in_=w1[:, 0, :])
            w2_sb = constp.tile([128, NT, HD], F32)
            nc.gpsimd.dma_start(out=w2_sb[:, 0, :], in_=w2[:, 0, :])
            mask_sb = constp.tile([128, 128], F32)
            nc.gpsimd.dma_start(out=mask_sb, in_=mask4[:, :])
            wo_sb = wpool.tile([128, HPC, D], WDT)
            nc.scalar.dma_start(out=wo_sb, in_=wo[:, :, :])
            eps_sb = constp.tile([128, 1], F32)
            nc.vector.memset(eps_sb, EPS)

            # persistent activations
            # v_sb: [kpos-part, t, head, 130]; col 128 is the ones column
            # feeding the softmax denominator, col 129 is alignment pad.
            # Fill with ones up front; phase-1 copies overwrite cols 0:128.
            v_sb = persist.tile([128, NT, HPC, VW], WDT)
            nc.vector.memset(v_sb, 1.0)
            qT = persist.tile([128, HPC, L], WDT)               # [d, h, L]
            kT = persist.tile([128, HPC, L], WDT)

            def phase1_tile(t, ps_pv):
                x_tile = xin.tile([128, NHC, LT], WDT, tag="x", name="x_tile")
                nc.gpsimd.dma_start(out=x_tile, in_=xt[t, :, :, :])

                p_qk = ps_pv.tile([128, 4 * HD], F32, tag="pqk", name="p_qk")
                p_v = ps_pv.tile([128, HPC * HD], F32, tag="pv", name="p_v")
                for c in range(NHC):
                    nc.tensor.matmul(p_qk, x_tile[:, c, :], w_sb[:, c, 0:4 * HD],
                                     start=(c == 0), stop=(c == NHC - 1))
                    nc.tensor.matmul(p_v, x_tile[:, c, :],
                                     w_sb[:, c, 4 * HD:6 * HD],
                                     start=(c == 0), stop=(c == NHC - 1))

                nc.scalar.copy(
                    v_sb[:, t, :, 0:HD],
                    p_v.rearrange("p (h d) -> p h d", h=HPC))

                # rms-norm scale: s = 1/sqrt(mean(x^2) + eps) per (L, seg)
                sq = qkvp.tile([128, 4 * HD], F32, tag="sq", name="sq")
                nc.scalar.activation(sq, p_qk, AF.Square)
                ssum = qkvp.tile([128, 4], F32, tag="ssum", name="ssum")
                nc.vector.reduce_sum(ssum, sq.rearrange("p (g d) -> p g d", g=4),
                                     axis=AX.X)
                nc.scalar.activation(ssum, ssum, AF.Sqrt, scale=1.0 / HD,
                                     bias=eps_sb)
                s_val = qkvp.tile([128, 4], F32, tag="sval", name="s_val")
                nc.vector.reciprocal(s_val, ssum)

                # rope (batched): qk_n = qk * s; z = qk_n .* W; pair-add
                qk_n = qkvp.tile([128, 4 * HD], F32, tag="qkn", name="qk_n")
                nc.vector.tensor_mul(qk_n.rearrange("p (g d) -> p g d", g=4),
                                     p_qk.rearrange("p (g d) -> p g d", g=4),
                                     s_val.to_broadcast([128, 4, HD]))
                roped = qkvp.tile([128, 4 * HD], WDT, tag="roped", name="roped")
                roped4 = roped.rearrange("p (g h x) -> p g h x", g=4, h=2)
                for half, wtab in ((0, w1_sb), (1, w2_sb)):
                    z = qkvp.tile([128, 4 * HD], F32, tag="z", name="z")
                    nc.vector.tensor_mul(z.rearrange("p (g d) -> p g d", g=4),
                                         qk_n.rearrange("p (g d) -> p g d", g=4),
                                         _bcast_mid(wtab[:, t, :], 4))
                    with nc.allow_low_precision("2-elem rope pairs"):
                        nc.vector.reduce_sum(
                            roped4[:, :, half, :],
                            z.rearrange("p (g x two) -> p g x two", g=4, two=2),
                            axis=AX.X)

                # xbar-transpose the 4 roped [128,128] blocks into qT/kT
                for seg in range(4):
                    tgt = qT if seg < 2 else kT
                    h = seg % 2
                    nc.sync.dma_start(
                        out=tgt[:, h, t * LT:(t + 1) * LT],
                        in_=roped[:, seg * HD:(seg + 1) * HD],
                        transpose=True)

            def attention_head(g, h, ps_qs, ps_po, oT_tiles):
                """Scores + exp + PV for one (q-group, head)."""
                nkc = 4 * g + 4
                p_o = ps_po.tile([128, 4, 512], F32, tag="po", name="p_o")
                exps = {}

                def score(kc):
                    diag = kc >= 4 * g
                    q0 = (kc - 4 * g) * 128 if diag else 0
                    p_s = ps_qs.tile([128, QT], F32, tag="qs", name="p_s")
                    nc.tensor.matmul(
                        p_s[:, q0:QT], kT[:, h, kc * 128:(kc + 1) * 128],
                        qT[:, h, g * QT + q0:(g + 1) * QT],
                        start=True, stop=True)
                    if diag:
                        nc.vector.tensor_add(
                            p_s[:, q0:q0 + 128], p_s[:, q0:q0 + 128], mask_sb)
                    expT = attnp.tile([128, QT], WDT, tag="expT", bufs=6,
                                      name="expT")
                    nc.scalar.activation(expT[:, q0:QT], p_s[:, q0:QT],
                                         AF.Exp, scale=SCALE)
                    exps[kc] = expT

                def pv(kc):
                    expT = exps.pop(kc)
                    for qc in range(max(0, kc - 4 * g), 4):
                        nc.tensor.matmul(
                            p_o[:, qc, 0:VW - 1],
                            expT[:, qc * 128:(qc + 1) * 128],
                            v_sb[:, kc, h, 0:VW - 1],
                            start=(kc == 0), stop=(kc == 4 * g + qc))

                # keep the score matmul one chunk ahead of PV on the PE queue
                score(0)
                for kc in range(1, nkc):
                    score(kc)
                    pv(kc - 1)
                pv(nkc - 1)

                # o_sb[q, vd] = p_o[:, qc, 0:128] / den;  den = col 128
                for qc in range(4):
                    inv = attnp.tile([128, 1], F32, tag="inv", bufs=4,
                                     name="inv")
                    nc.vector.reciprocal(inv, p_o[:, qc, 128:129])
                    o_sb = attnp.tile([128, HD], WDT, tag="osb", bufs=8,
                                      name="o_sb")
                    nc.scalar.activation(o_sb, p_o[:, qc, 0:HD], AF.Copy,
                                         scale=inv)
                    oT = resp.tile([128, 128], WDT, tag="oT", bufs=16,
                                   name="oT")
                    nc.sync.dma_start(out=oT, in_=o_sb, transpose=True)
                    oT_tiles[(h, qc)] = oT

            def out_proj(g, ps_py, oT_tiles):
                for tt in range(4):
                    t = 4 * g + tt
                    for ec in range(4):
                        p_y = ps_py.tile([128, QT], F32, tag="py", name="p_y")
                        nc.tensor.matmul(p_y, oT_tiles[(0, tt)],
                                         wo_sb[:, 0, ec * 512:(ec + 1) * 512],
                                         start=True, stop=False)
                        nc.tensor.matmul(p_y, oT_tiles[(1, tt)],
                                         wo_sb[:, 1, ec * 512:(ec + 1) * 512],
                                         start=False, stop=True)
                        y = resp.tile([128, QT], F32, tag="y", bufs=4,
                                      name="y")
                        if ec == 0:
                            nc.scalar.copy(y, p_y)
                        else:
                            nc.vector.tensor_copy(y, p_y)
                        nc.gpsimd.dma_start(
                            out=out[t * LT:(t + 1) * LT,
                                    ec * 512:(ec + 1) * 512],
                            in_=y)

            with tc.tile_pool(name="ps_qkv", bufs=2, space="PSUM") as ps_pv:
                phase1_tile(0, ps_pv)
                # bulk rope-table load: scalar queue, split so early tiles
                # aren't gated behind the whole table
                nc.scalar.dma_start(out=w1_sb[:, 1:4, :], in_=w1[:, 1:4, :])
                nc.scalar.dma_start(out=w2_sb[:, 1:4, :], in_=w2[:, 1:4, :])
                nc.scalar.dma_start(out=w1_sb[:, 4:NT, :], in_=w1[:, 4:NT, :])
                nc.scalar.dma_start(out=w2_sb[:, 4:NT, :], in_=w2[:, 4:NT, :])
                for t in range(1, NT):
                    phase1_tile(t, ps_pv)
            with (
                tc.tile_pool(name="ps_s", bufs=2, space="PSUM") as ps_qs,
                tc.tile_pool(name="ps_o", bufs=1, space="PSUM") as ps_po,
                tc.tile_pool(name="ps_y", bufs=2, space="PSUM") as ps_py,
            ):
                # out_proj(g) is deferred until after group g+1's attention so
                # the PE isn't stalled on g's copy-scale -> xbar-transpose chain
                prev = None
                for g in range(NQT):
                    oT_tiles = {}
                    attention_head(g, 0, ps_qs, ps_po, oT_tiles)
                    attention_head(g, 1, ps_qs, ps_po, oT_tiles)
                    if prev is not None:
                        out_proj(prev[0], ps_py, prev[1])
                    prev = (g, oT_tiles)
                out_proj(prev[0], ps_py, prev[1])
    nc.compile()
    return nc


_NC_CACHE = None


def _get_nc():
    global _NC_CACHE
    if _NC_CACHE is None:
        _NC_CACHE = build()
    return _NC_CACHE


def prep_inputs(x, w_qkv, w_out):
    """Host-side sharding/layout prep. Returns list of per-core input maps."""
    wnp = ml_dtypes.bfloat16 if WDTYPE == "bf16" else np.float32
    x2d = np.asarray(x, dtype=np.float32).reshape(L, D)
    w_qkv = np.asarray(w_qkv, dtype=np.float32)
    w_out = np.asarray(w_out, dtype=np.float32)

    # xt[t, c, p, l] = x2d[t*128 + l, c*128 + p]
    # [t, p(hid), c, l] so each per-tile DMA is one linear stream
    xt = np.ascontiguousarray(
        x2d.reshape(NT, LT, NHC, HC).transpose(0, 3, 2, 1)).astype(wnp)

    # rope coefficient tables
    inv_freq = 1.0 / (ROPE_BASE ** (np.arange(0, HD, 2, dtype=np.float64) / HD))
    pos = np.arange(L, dtype=np.float64)[:, None]
    ang = pos * inv_freq[None, :]                       # [L, 64]
    cos, sin = np.cos(ang), np.sin(ang)
    w1 = np.zeros((L, HD), dtype=np.float32)
    w2 = np.zeros((L, HD), dtype=np.float32)
    w1[:, 0::2] = -sin
    w1[:, 1::2] = cos
    w2[:, 0::2] = cos
    w2[:, 1::2] = sin
    w1 = np.ascontiguousarray(w1.reshape(NT, LT, HD).transpose(1, 0, 2))
    w2 = np.ascontiguousarray(w2.reshape(NT, LT, HD).transpose(1, 0, 2))

    # causal mask tile for diagonal blocks
    i = np.arange(128)[:, None]
    j = np.arange(128)[None, :]
    mask4 = np.where(i <= j, 0.0, NEG).astype(np.float32)  # [128, 128]

    in_maps = []
    for c in range(N_CORES):
        h0 = HPC * c
        rows = []
        for part in range(3):  # q, k, v
            for hh in range(HPC):
                base = part * D + (h0 + hh) * HD
                rows.append(w_qkv[base:base + HD])
        w_c = np.concatenate(rows, axis=0)              # [768, D]
        wt = np.ascontiguousarray(w_c.T).astype(wnp)    # [D, 768]
        wo = np.ascontiguousarray(
            w_out[:, h0 * HD:(h0 + HPC) * HD].T.reshape(HPC, HD, D)
            .transpose(1, 0, 2)).astype(wnp)
        in_maps.append({
            "xt": xt, "wt": wt, "wo": wo, "w1": w1, "w2": w2,
            "mask4": mask4,
        })
    return in_maps


def kernel(x, w_qkv, w_out, mask, _trace=False):
    """Full MHA forward. Returns [1, L, D] float32."""
    nc = _get_nc()
    in_maps = prep_inputs(x, w_qkv, w_out)
    res = run_bass_kernel_spmd(nc, in_maps, core_ids=list(range(N_CORES)),
                               trace=_trace)
    acc = np.zeros((L, D), dtype=np.float32)
    for r in res.results:
        acc += r["out"]
    out = acc.reshape(1, L, D)
    if _trace:
        return out, res
    return out


# revision 12
# speedup vs baseline: 1.3274x; 1.3274x over previous
"""Trainium2 Bass kernel for 16-head causal MHA (RMSNorm+RoPE on q,k).

Tensor-parallel over heads: 8 cores x 2 heads each. Each core computes
qkv projection for its heads, norm+rope, causal attention, and a partial
out-projection; the host sums the 8 partial outputs.

v2 layout notes (vs the original v-stationary design):
- Scores are computed transposed [k, q]; exp tiles then serve as the
  STATIONARY matmul operand for PV, with a ones-column appended to V, so
  the PV output lands as [q, v|den] in PSUM: the softmax denominator is
  column 128 and the division becomes a per-partition ACT copy-scale.
  This removes all denominator matmuls and all per-column scaling ops.
- All 128x128 transposes (phase-1 q/k blocks, phase-2 o blocks) go
  through the DMA xbar transpose engine instead of TensorE.
- RoPE is computed into a de-interleaved [odd-half | even-half] d-order,
  identically for q and k (dot products unchanged); v / out_proj keep the
  natural d-order.
- exp() is computed without max-subtraction: post-RMSNorm |q.k|/sqrt(hd)
  <= sqrt(128), so exp is bounded by ~8.2e4. Masked (upper-triangular)
  score blocks are skipped entirely; diagonal blocks get an additive -1e9.
"""
import os
import ml_dtypes
import numpy as np

import concourse.bacc as bacc
import concourse.mybir as mybir
import concourse.tile as tile
from concourse.ap import AP
from concourse.bass_utils import run_bass_kernel_spmd


def _bcast_mid(ap2d, n):
    """[128, X] -> [128, n, X] with step-0 middle dim."""
    return AP(tensor=ap2d.tensor, offset=ap2d.offset,
              ap=[list(ap2d.ap[0]), [0, n], list(ap2d.ap[1])])

F32 = mybir.dt.float32
F32R = mybir.dt.float32r
BF16 = mybir.dt.bfloat16
WDTYPE = os.environ.get("MHA_WDTYPE", "bf16")
WDT = BF16 if WDTYPE == "bf16" else F32R
AF = mybir.ActivationFunctionType
ALU = mybir.AluOpType
AX = mybir.AxisListType

N_CORES = 8
L = 2048
D = 2048
HD = 128
N_HEAD = 16
HPC = N_HEAD // N_CORES  # heads per core = 2
LT = 128                 # L-tile rows
NT = L // LT             # 16 L-tiles
HC = 128                 # hid chunk
NHC = D // HC            # 16 hid chunks
QT = 512                 # q-tile width in attention
NQT = L // QT            # 4
VW = 130                 # v row stride (128 dims + ones col + pad)
EPS = 1e-5
ROPE_BASE = 10000.0
SCALE = 1.0 / float(np.sqrt(HD))
NEG = -1.0e9


def build():
    nc = bacc.Bacc("TRN2", target_bir_lowering=False, debug=False,
                   enable_asserts=False, num_devices=N_CORES)

    # Per-core external inputs (host-prepped layouts; see prep_inputs()).
    xt = nc.dram_tensor("xt", [NT, HC, NHC, LT], WDT, kind="ExternalInput")
    wt = nc.dram_tensor("wt", [D, 6 * HD], WDT, kind="ExternalInput")
    wo = nc.dram_tensor("wo", [HD, HPC, D], WDT, kind="ExternalInput")
    w1 = nc.dram_tensor("w1", [LT, NT, HD], F32, kind="ExternalInput")
    w2 = nc.dram_tensor("w2", [LT, NT, HD], F32, kind="ExternalInput")
    mask4 = nc.dram_tensor("mask4", [128, 128], F32, kind="ExternalInput")
    ident_in = nc.dram_tensor("ident", [128, 128], WDT, kind="ExternalInput")

    out = nc.dram_tensor("out", [L, D], F32, kind="ExternalOutput")

    with tile.TileContext(nc) as tc:
        with (
            tc.tile_pool(name="const", bufs=1) as constp,
            tc.tile_pool(name="wpool", bufs=1) as wpool,
            tc.tile_pool(name="persist", bufs=1) as persist,
            tc.tile_pool(name="xin", bufs=3) as xin,
            tc.tile_pool(name="qkv", bufs=3) as qkvp,
            tc.tile_pool(name="attn", bufs=4) as attnp,
            tc.tile_pool(name="res", bufs=4) as resp,
        ):
            # ---- weights resident (w chunks first: they gate the GEMMs).
            # Split across the two HWDGE queues for startup bandwidth.
            w_sb = wpool.tile([128, NHC, 6 * HD], WDT)
            for c in range(NHC):
                eng = nc.sync if c % 2 == 0 else nc.scalar
                eng.dma_start(out=w_sb[:, c, :],
                              in_=wt[c * 128:(c + 1) * 128, :])
            w1_sb = constp.tile([128, NT, HD], F32)
            nc.gpsimd.dma_start(out=w1_sb[:, 0, :], in_=w1[:, 0, :])
            w2_sb = constp.tile([128, NT, HD], F32)
            nc.gpsimd.dma_start(out=w2_sb[:, 0, :], in_=w2[:, 0, :])
            mask_sb = constp.tile([128, 128], F32)
            nc.gpsimd.dma_start(out=mask_sb, in_=mask4[:, :])
            ident = constp.tile([128, 128], WDT)
            nc.gpsimd.dma_start(out=ident, in_=ident_in[:, :])
            wo_sb = wpool.tile([128, HPC, D], WDT)
            eps_sb = constp.tile([128, 1], F32)
            nc.vector.memset(eps_sb, EPS)

            # persistent activations
            # v_sb: [kpos-part, t, head, 130]; col 128 is the ones column
            # feeding the softmax denominator, col 129 is alignment pad.
            # Fill with ones up front; phase-1 copies overwrite cols 0:128.
            v_sb = persist.tile([128, NT, HPC, VW], WDT)
            nc.vector.memset(v_sb, 1.0)
            qT = persist.tile([128, HPC, L], WDT)               # [d, h, L]
            kT = persist.tile([128, HPC, L], WDT)

            def phase1_tile(t, ps_pv, ps_tp):
                x_tile = xin.tile([128, NHC, LT], WDT, tag="x", name="x_tile")
                nc.gpsimd.dma_start(out=x_tile, in_=xt[t, :, :, :])

                p_qk = ps_pv.tile([128, 4 * HD], F32, tag="pqk", name="p_qk")
                p_v = ps_pv.tile([128, HPC * HD], F32, tag="pv", name="p_v")
                for c in range(NHC):
                    nc.tensor.matmul(p_qk, x_tile[:, c, :], w_sb[:, c, 0:4 * HD],
                                     start=(c == 0), stop=(c == NHC - 1))
                    nc.tensor.matmul(p_v, x_tile[:, c, :],
                                     w_sb[:, c, 4 * HD:6 * HD],
                                     start=(c == 0), stop=(c == NHC - 1))

                nc.scalar.copy(
                    v_sb[:, t, :, 0:HD],
                    p_v.rearrange("p (h d) -> p h d", h=HPC))

                # rms-norm scale: s = 1/sqrt(mean(x^2) + eps) per (L, seg)
                sq = qkvp.tile([128, 4 * HD], F32, tag="sq", name="sq")
                nc.scalar.activation(sq, p_qk, AF.Square)
                ssum = qkvp.tile([128, 4], F32, tag="ssum", name="ssum")
                nc.vector.reduce_sum(ssum, sq.rearrange("p (g d) -> p g d", g=4),
                                     axis=AX.X)
                nc.scalar.activation(ssum, ssum, AF.Sqrt, scale=1.0 / HD,
                                     bias=eps_sb)
                s_val = qkvp.tile([128, 4], F32, tag="sval", name="s_val")
                nc.vector.reciprocal(s_val, ssum)

                # rope (batched): qk_n = qk * s; z = qk_n .* W; pair-add
                qk_n = qkvp.tile([128, 4 * HD], F32, tag="qkn", name="qk_n")
                nc.vector.tensor_mul(qk_n.rearrange("p (g d) -> p g d", g=4),
                                     p_qk.rearrange("p (g d) -> p g d", g=4),
                                     s_val.to_broadcast([128, 4, HD]))
                roped = qkvp.tile([128, 4 * HD], WDT, tag="roped", name="roped")
                roped4 = roped.rearrange("p (g h x) -> p g h x", g=4, h=2)
                for half, wtab in ((0, w1_sb), (1, w2_sb)):
                    z = qkvp.tile([128, 4 * HD], F32, tag="z", name="z")
                    nc.vector.tensor_mul(z.rearrange("p (g d) -> p g d", g=4),
                                         qk_n.rearrange("p (g d) -> p g d", g=4),
                                         _bcast_mid(wtab[:, t, :], 4))
                    with nc.allow_low_precision("2-elem rope pairs"):
                        nc.vector.reduce_sum(
                            roped4[:, :, half, :],
                            z.rearrange("p (g x two) -> p g x two", g=4, two=2),
                            axis=AX.X)

                # transpose the 4 roped [128,128] blocks into qT/kT on PE
                for seg in range(4):
                    tgt = qT if seg < 2 else kT
                    h = seg % 2
                    p_tr = ps_tp.tile([128, 128], WDT, tag="tp", name="p_tr")
                    nc.tensor.transpose(
                        p_tr, roped[:, seg * HD:(seg + 1) * HD], ident)
                    nc.scalar.copy(tgt[:, h, t * LT:(t + 1) * LT], p_tr)

            def attention_head(g, h, ps_qs, ps_po, oT_tiles):
                """Scores + exp + PV for one (q-group, head)."""
                nkc = 4 * g + 4
                p_o = ps_po.tile([128, 4, 512], F32, tag="po", name="p_o")
                exps = {}

                def score(kc):
                    diag = kc >= 4 * g
                    q0 = (kc - 4 * g) * 128 if diag else 0
                    p_s = ps_qs.tile([128, QT], F32, tag="qs", name="p_s")
                    nc.tensor.matmul(
                        p_s[:, q0:QT], kT[:, h, kc * 128:(kc + 1) * 128],
                        qT[:, h, g * QT + q0:(g + 1) * QT],
                        start=True, stop=True)
                    if diag:
                        nc.vector.tensor_add(
                            p_s[:, q0:q0 + 128], p_s[:, q0:q0 + 128], mask_sb)
                    expT = attnp.tile([128, QT], WDT, tag="expT", bufs=6,
                                      name="expT")
                    nc.scalar.activation(expT[:, q0:QT], p_s[:, q0:QT],
                                         AF.Exp, scale=SCALE)
                    exps[kc] = expT

                def pv(kc):
                    expT = exps.pop(kc)
                    for qc in range(max(0, kc - 4 * g), 4):
                        nc.tensor.matmul(
                            p_o[:, qc, 0:VW - 1],
                            expT[:, qc * 128:(qc + 1) * 128],
                            v_sb[:, kc, h, 0:VW - 1],
                            start=(kc == 0), stop=(kc == 4 * g + qc))

                # keep the score matmul one chunk ahead of PV on the PE queue
                score(0)
                for kc in range(1, nkc):
                    score(kc)
                    pv(kc - 1)
                pv(nkc - 1)

                # o_sb[q, vd] = p_o[:, qc, 0:128] / den;  den = col 128
                for qc in range(4):
                    inv = attnp.tile([128, 1], F32, tag="inv", bufs=4,
                                     name="inv")
                    nc.vector.reciprocal(inv, p_o[:, qc, 128:129])
                    o_sb = attnp.tile([128, HD], WDT, tag="osb", bufs=8,
                                      name="o_sb")
                    nc.scalar.activation(o_sb, p_o[:, qc, 0:HD], AF.Copy,
                                         scale=inv)
                    oT = resp.tile([128, 128], WDT, tag="oT", bufs=16,
                                   name="oT")
                    nc.sync.dma_start(out=oT, in_=o_sb, transpose=True)
                    oT_tiles[(h, qc)] = oT

            def out_proj(g, ps_py, oT_tiles):
                for tt in range(4):
                    t = 4 * g + tt
                    for ec in range(4):
                        p_y = ps_py.tile([128, QT], F32, tag="py", name="p_y")
                        nc.tensor.matmul(p_y, oT_tiles[(0, tt)],
                                         wo_sb[:, 0, ec * 512:(ec + 1) * 512],
                                         start=True, stop=False)
                        nc.tensor.matmul(p_y, oT_tiles[(1, tt)],
                                         wo_sb[:, 1, ec * 512:(ec + 1) * 512],
                                         start=False, stop=True)
                        y = resp.tile([128, QT], F32, tag="y", bufs=4,
                                      name="y")
                        if ec == 0:
                            nc.scalar.copy(y, p_y)
                        else:
                            nc.vector.tensor_copy(y, p_y)
                        eng = nc.gpsimd if ec % 2 == 0 else nc.sync
                        eng.dma_start(
                            out=out[t * LT:(t + 1) * LT,
                                    ec * 512:(ec + 1) * 512],
                            in_=y)

            with (
                tc.tile_pool(name="ps_qkv", bufs=2, space="PSUM") as ps_pv,
                tc.tile_pool(name="ps_tr", bufs=2, space="PSUM") as ps_tp1,
            ):
                phase1_tile(0, ps_pv, ps_tp1)
                # bulk rope-table load: scalar queue, split so early tiles
                # aren't gated behind the whole table
                nc.scalar.dma_start(out=w1_sb[:, 1:4, :], in_=w1[:, 1:4, :])
                nc.scalar.dma_start(out=w2_sb[:, 1:4, :], in_=w2[:, 1:4, :])
                nc.scalar.dma_start(out=w1_sb[:, 4:NT, :], in_=w1[:, 4:NT, :])
                nc.scalar.dma_start(out=w2_sb[:, 4:NT, :], in_=w2[:, 4:NT, :])
                nc.scalar.dma_start(out=wo_sb, in_=wo[:, :, :])
                for t in range(1, NT):
                    phase1_tile(t, ps_pv, ps_tp1)
            with (
                tc.tile_pool(name="ps_s", bufs=2, space="PSUM") as ps_qs,
                tc.tile_pool(name="ps_o", bufs=1, space="PSUM") as ps_po,
                tc.tile_pool(name="ps_y", bufs=2, space="PSUM") as ps_py,
            ):
                # out_proj(g) is deferred until after group g+1's first head so
                # the PE isn't stalled on g's copy-scale -> xbar-transpose chain
                prev = None
                for g in range(NQT):
                    oT_tiles = {}
                    attention_head(g, 0, ps_qs, ps_po, oT_tiles)
                    if prev is not None:
                        out_proj(prev[0], ps_py, prev[1])
                    attention_head(g, 1, ps_qs, ps_po, oT_tiles)
                    prev = (g, oT_tiles)
                out_proj(prev[0], ps_py, prev[1])
    nc.compile()
    return nc


_NC_CACHE = None


def _get_nc():
    global _NC_CACHE
    if _NC_CACHE is None:
        _NC_CACHE = build()
    return _NC_CACHE


def prep_inputs(x, w_qkv, w_out):
    """Host-side sharding/layout prep. Returns list of per-core input maps."""
    wnp = ml_dtypes.bfloat16 if WDTYPE == "bf16" else np.float32
    x2d = np.asarray(x, dtype=np.float32).reshape(L, D)
    w_qkv = np.asarray(w_qkv, dtype=np.float32)
    w_out = np.asarray(w_out, dtype=np.float32)

    # xt[t, c, p, l] = x2d[t*128 + l, c*128 + p]
    # [t, p(hid), c, l] so each per-tile DMA is one linear stream
    xt = np.ascontiguousarray(
        x2d.reshape(NT, LT, NHC, HC).transpose(0, 3, 2, 1)).astype(wnp)

    # rope coefficient tables
    inv_freq = 1.0 / (ROPE_BASE ** (np.arange(0, HD, 2, dtype=np.float64) / HD))
    pos = np.arange(L, dtype=np.float64)[:, None]
    ang = pos * inv_freq[None, :]                       # [L, 64]
    cos, sin = np.cos(ang), np.sin(ang)
    w1 = np.zeros((L, HD), dtype=np.float32)
    w2 = np.zeros((L, HD), dtype=np.float32)
    w1[:, 0::2] = -sin
    w1[:, 1::2] = cos
    w2[:, 0::2] = cos
    w2[:, 1::2] = sin
    w1 = np.ascontiguousarray(w1.reshape(NT, LT, HD).transpose(1, 0, 2))
    w2 = np.ascontiguousarray(w2.reshape(NT, LT, HD).transpose(1, 0, 2))

    # causal mask tile for diagonal blocks
    i = np.arange(128)[:, None]
    j = np.arange(128)[None, :]
    mask4 = np.where(i <= j, 0.0, NEG).astype(np.float32)  # [128, 128]
    ident = np.eye(128, dtype=np.float32).astype(wnp)

    in_maps = []
    for c in range(N_CORES):
        h0 = HPC * c
        rows = []
        for part in range(3):  # q, k, v
            for hh in range(HPC):
                base = part * D + (h0 + hh) * HD
                rows.append(w_qkv[base:base + HD])
        w_c = np.concatenate(rows, axis=0)              # [768, D]
        wt = np.ascontiguousarray(w_c.T).astype(wnp)    # [D, 768]
        wo = np.ascontiguousarray(
            w_out[:, h0 * HD:(h0 + HPC) * HD].T.reshape(HPC, HD, D)
            .transpose(1, 0, 2)).astype(wnp)
        in_maps.append({
            "xt": xt, "wt": wt, "wo": wo, "w1": w1, "w2": w2,
            "mask4": mask4, "ident": ident,
        })
    return in_maps


def kernel(x, w_qkv, w_out, mask, _trace=False):
    """Full MHA forward. Returns [1, L, D] float32."""
    nc = _get_nc()
    in_maps = prep_inputs(x, w_qkv, w_out)
    res = run_bass_kernel_spmd(nc, in_maps, core_ids=list(range(N_CORES)),
                               trace=_trace)
    acc = np.zeros((L, D), dtype=np.float32)
    for r in res.results:
        acc += r["out"]
    out = acc.reshape(1, L, D)
    if _trace:
        return out, res
    return out


# revision 15
# speedup vs baseline: 1.4154x; 1.0663x over previous
"""Trainium2 Bass kernel for 16-head causal MHA (RMSNorm+RoPE on q,k).

Tensor-parallel over heads: 8 cores x 2 heads each. Each core computes
qkv projection for its heads, norm+rope, causal attention, and a partial
out-projection; the host sums the 8 partial outputs.

v2 layout notes (vs the original v-stationary design):
- Scores are computed transposed [k, q]; exp tiles then serve as the
  STATIONARY matmul operand for PV, with a ones-column appended to V, so
  the PV output lands as [q, v|den] in PSUM: the softmax denominator is
  column 128 and the division becomes a per-partition ACT copy-scale.
  This removes all denominator matmuls and all per-column scaling ops.
- All 128x128 transposes (phase-1 q/k blocks, phase-2 o blocks) go
  through the DMA xbar transpose engine instead of TensorE.
- RoPE is computed into a de-interleaved [odd-half | even-half] d-order,
  identically for q and k (dot products unchanged); v / out_proj keep the
  natural d-order.
- exp() is computed without max-subtraction: post-RMSNorm |q.k|/sqrt(hd)
  <= sqrt(128), so exp is bounded by ~8.2e4. Masked (upper-triangular)
  score blocks are skipped entirely; diagonal blocks get an additive -1e9.
"""
import os
import ml_dtypes
import numpy as np

import concourse.bacc as bacc
import concourse.mybir as mybir
import concourse.tile as tile
from concourse.ap import AP
from concourse.bass_utils import run_bass_kernel_spmd


def _bcast_mid(ap2d, n):
    """[128, X] -> [128, n, X] with step-0 middle dim."""
    return AP(tensor=ap2d.tensor, offset=ap2d.offset,
              ap=[list(ap2d.ap[0]), [0, n], list(ap2d.ap[1])])

F32 = mybir.dt.float32
F32R = mybir.dt.float32r
BF16 = mybir.dt.bfloat16
WDTYPE = os.environ.get("MHA_WDTYPE", "bf16")
WDT = BF16 if WDTYPE == "bf16" else F32R
AF = mybir.ActivationFunctionType
ALU = mybir.AluOpType
AX = mybir.AxisListType

N_CORES = 8
L = 2048
D = 2048
HD = 128
N_HEAD = 16
HPC = N_HEAD // N_CORES  # heads per core = 2
LT = 128                 # L-tile rows
NT = L // LT             # 16 L-tiles
HC = 128                 # hid chunk
NHC = D // HC            # 16 hid chunks
QT = 512                 # q-tile width in attention
NQT = L // QT            # 4
VW = 130                 # v row stride (128 dims + ones col + pad)
EPS = 1e-5
ROPE_BASE = 10000.0
SCALE = 1.0 / float(np.sqrt(HD))
NEG = -1.0e9


def build():
    nc = bacc.Bacc("TRN2", target_bir_lowering=False, debug=False,
                   enable_asserts=False, num_devices=N_CORES)

    # Per-core external inputs (host-prepped layouts; see prep_inputs()).
    xt = nc.dram_tensor("xt", [NT, HC, NHC, LT], WDT, kind="ExternalInput")
    wt = nc.dram_tensor("wt", [D, 6 * HD], WDT, kind="ExternalInput")
    wo = nc.dram_tensor("wo", [HD, HPC, D], WDT, kind="ExternalInput")
    w1 = nc.dram_tensor("w1", [LT, NT, HD], F32, kind="ExternalInput")
    w2 = nc.dram_tensor("w2", [LT, NT, HD], F32, kind="ExternalInput")
    mask4 = nc.dram_tensor("mask4", [128, 128], F32, kind="ExternalInput")
    ident_in = nc.dram_tensor("ident", [128, 128], WDT, kind="ExternalInput")

    out = nc.dram_tensor("out", [L, D], F32, kind="ExternalOutput")

    with tile.TileContext(nc) as tc:
        with (
            tc.tile_pool(name="const", bufs=1) as constp,
            tc.tile_pool(name="wpool", bufs=1) as wpool,
            tc.tile_pool(name="persist", bufs=1) as persist,
            tc.tile_pool(name="xin", bufs=3) as xin,
            tc.tile_pool(name="qkv", bufs=3) as qkvp,
            tc.tile_pool(name="attn", bufs=4) as attnp,
            tc.tile_pool(name="res", bufs=4) as resp,
        ):
            # ---- weights resident (w chunks first: they gate the GEMMs).
            # Split across the two HWDGE queues for startup bandwidth.
            w_sb = wpool.tile([128, NHC, 6 * HD], WDT)
            for c in range(NHC):
                eng = nc.sync if c % 2 == 0 else nc.scalar
                eng.dma_start(out=w_sb[:, c, :],
                              in_=wt[c * 128:(c + 1) * 128, :])
            w1_sb = constp.tile([128, NT, HD], F32)
            nc.gpsimd.dma_start(out=w1_sb[:, 0, :], in_=w1[:, 0, :])
            w2_sb = constp.tile([128, NT, HD], F32)
            nc.gpsimd.dma_start(out=w2_sb[:, 0, :], in_=w2[:, 0, :])
            mask_sb = constp.tile([128, 128], F32)
            nc.gpsimd.dma_start(out=mask_sb, in_=mask4[:, :])
            ident = constp.tile([128, 128], WDT)
            nc.gpsimd.dma_start(out=ident, in_=ident_in[:, :])
            wo_sb = wpool.tile([128, HPC, D], WDT)
            eps_sb = constp.tile([128, 1], F32)
            nc.vector.memset(eps_sb, EPS)

            # persistent activations
            # v_sb: [kpos-part, t, head, 130]; col 128 is the ones column
            # feeding the softmax denominator, col 129 is alignment pad.
            # Fill with ones up front; phase-1 copies overwrite cols 0:128.
            v_sb = persist.tile([128, NT, HPC, VW], WDT)
            nc.vector.memset(v_sb, 1.0)
            qT = persist.tile([128, HPC, L], WDT)               # [d, h, L]
            kT = persist.tile([128, HPC, L], WDT)

            def phase1_tile(t, ps_pv, ps_tp):
                x_tile = xin.tile([128, NHC, LT], WDT, tag="x", name="x_tile")
                nc.gpsimd.dma_start(out=x_tile, in_=xt[t, :, :, :])

                p_qk = ps_pv.tile([128, 4 * HD], F32, tag="pqk", name="p_qk")
                p_v = ps_pv.tile([128, HPC * HD], F32, tag="pv", name="p_v")
                for c in range(NHC):
                    nc.tensor.matmul(p_qk, x_tile[:, c, :], w_sb[:, c, 0:4 * HD],
                                     start=(c == 0), stop=(c == NHC - 1))
                    nc.tensor.matmul(p_v, x_tile[:, c, :],
                                     w_sb[:, c, 4 * HD:6 * HD],
                                     start=(c == 0), stop=(c == NHC - 1))

                nc.scalar.copy(
                    v_sb[:, t, :, 0:HD],
                    p_v.rearrange("p (h d) -> p h d", h=HPC))

                # rms-norm scale: s = 1/sqrt(mean(x^2) + eps) per (L, seg)
                sq = qkvp.tile([128, 4 * HD], F32, tag="sq", name="sq")
                nc.scalar.activation(sq, p_qk, AF.Square)
                ssum = qkvp.tile([128, 4], F32, tag="ssum", name="ssum")
                nc.vector.reduce_sum(ssum, sq.rearrange("p (g d) -> p g d", g=4),
                                     axis=AX.X)
                nc.scalar.activation(ssum, ssum, AF.Sqrt, scale=1.0 / HD,
                                     bias=eps_sb)
                s_val = qkvp.tile([128, 4], F32, tag="sval", name="s_val")
                nc.vector.reciprocal(s_val, ssum)

                # rope (batched): qk_n = qk * s; z = qk_n .* W; pair-add
                qk_n = qkvp.tile([128, 4 * HD], F32, tag="qkn", name="qk_n")
                nc.vector.tensor_mul(qk_n.rearrange("p (g d) -> p g d", g=4),
                                     p_qk.rearrange("p (g d) -> p g d", g=4),
                                     s_val.to_broadcast([128, 4, HD]))
                roped = qkvp.tile([128, 4 * HD], WDT, tag="roped", name="roped")
                roped4 = roped.rearrange("p (g h x) -> p g h x", g=4, h=2)
                for half, wtab in ((0, w1_sb), (1, w2_sb)):
                    z = qkvp.tile([128, 4 * HD], F32, tag="z", name="z")
                    nc.vector.tensor_mul(z.rearrange("p (g d) -> p g d", g=4),
                                         qk_n.rearrange("p (g d) -> p g d", g=4),
                                         _bcast_mid(wtab[:, t, :], 4))
                    with nc.allow_low_precision("2-elem rope pairs"):
                        nc.vector.reduce_sum(
                            roped4[:, :, half, :],
                            z.rearrange("p (g x two) -> p g x two", g=4, two=2),
                            axis=AX.X)

                # q blocks transpose on PE; k blocks (latency-tolerant: only
                # needed in phase 2) go through the xbar DMA transpose engine
                for seg in range(2):
                    p_tr = ps_tp.tile([128, 128], WDT, tag="tp", name="p_tr")
                    nc.tensor.transpose(
                        p_tr, roped[:, seg * HD:(seg + 1) * HD], ident)
                    nc.scalar.copy(qT[:, seg, t * LT:(t + 1) * LT], p_tr)
                for seg in range(2, 4):
                    nc.sync.dma_start(
                        out=kT[:, seg - 2, t * LT:(t + 1) * LT],
                        in_=roped[:, seg * HD:(seg + 1) * HD],
                        transpose=True)

            # p_o packs 4 q-chunks of [v(128)|den(1)] into 2 PSUM banks.
            # qc0/qc3's start=True clears their bank's has_written bits; qc1/2
            # then overwrite-on-clear-bit at kc==0 (PE FIFO runs qc0 first).
            PO_OFF = (0, 130, 260, 512)

            def attention_head(g, h, ps_qs, ps_po, ps_tp2, oT_tiles, filler):
                """Scores + exp + PV for one (q-group, head)."""
                nkc = 4 * g + 4
                p_o = ps_po.tile([128, 1024], F32, tag="po", name="p_o")
                exps = {}

                def score(kc):
                    diag = kc >= 4 * g
                    q0 = (kc - 4 * g) * 128 if diag else 0
                    p_s = ps_qs.tile([128, QT], F32, tag="qs", name="p_s")
                    nc.tensor.matmul(
                        p_s[:, q0:QT], kT[:, h, kc * 128:(kc + 1) * 128],
                        qT[:, h, g * QT + q0:(g + 1) * QT],
                        start=True, stop=True)
                    if diag:
                        nc.vector.tensor_add(
                            p_s[:, q0:q0 + 128], p_s[:, q0:q0 + 128], mask_sb)
                    expT = attnp.tile([128, QT], WDT, tag="expT", bufs=6,
                                      name="expT")
                    nc.scalar.activation(expT[:, q0:QT], p_s[:, q0:QT],
                                         AF.Exp, scale=SCALE)
                    exps[kc] = expT

                def pv(kc):
                    expT = exps.pop(kc)
                    for qc in range(max(0, kc - 4 * g), 4):
                        off = PO_OFF[qc]
                        nc.tensor.matmul(
                            p_o[:, off:off + 129],
                            expT[:, qc * 128:(qc + 1) * 128],
                            v_sb[:, kc, h, 0:129],
                            start=(kc == 0 and qc in (0, 3)),
                            stop=(kc == 4 * g + qc))

                def unit():
                    u = next(filler, None)
                    if u is not None:
                        u()

                # keep the score matmul one chunk ahead of PV on the PE queue;
                # interleave one out-proj unit of the previous group per step
                score(0)
                for kc in range(1, nkc):
                    score(kc)
                    pv(kc - 1)
                    unit()
                pv(nkc - 1)
                unit()

                # o_sb[q, vd] = p_o[q, 0:128] / den (den = col 128), then
                # transpose back to [vd, q] on PE for the out-projection
                for qc in range(4):
                    off = PO_OFF[qc]
                    inv = attnp.tile([128, 1], F32, tag="inv", bufs=4,
                                     name="inv")
                    nc.vector.reciprocal(inv, p_o[:, off + 128:off + 129])
                    o_sb = attnp.tile([128, HD], WDT, tag="osb", bufs=8,
                                      name="o_sb")
                    nc.scalar.activation(o_sb, p_o[:, off:off + HD], AF.Copy,
                                         scale=inv)
                    p_tr = ps_tp2.tile([128, 128], WDT, tag="tp2", name="p_tr2")
                    nc.tensor.transpose(p_tr, o_sb, ident)
                    oT = resp.tile([128, 128], WDT, tag="oT", bufs=16,
                                   name="oT")
                    nc.scalar.copy(oT, p_tr)
                    oT_tiles[(h, qc)] = oT

            def out_proj_units(g, ps_py, oT_tiles):
                for tt in range(4):
                    t = 4 * g + tt
                    for ec in range(4):
                        def u(t=t, tt=tt, ec=ec):
                            p_y = ps_py.tile([128, QT], F32, tag="py",
                                             name="p_y")
                            nc.tensor.matmul(
                                p_y, oT_tiles[(0, tt)],
                                wo_sb[:, 0, ec * 512:(ec + 1) * 512],
                                start=True, stop=False)
                            nc.tensor.matmul(
                                p_y, oT_tiles[(1, tt)],
                                wo_sb[:, 1, ec * 512:(ec + 1) * 512],
                                start=False, stop=True)
                            y = resp.tile([128, QT], F32, tag="y", bufs=4,
                                          name="y")
                            nc.vector.tensor_copy(y, p_y)
                            nc.gpsimd.dma_start(
                                out=out[t * LT:(t + 1) * LT,
                                        ec * 512:(ec + 1) * 512],
                                in_=y)
                        yield u

            with (
                tc.tile_pool(name="ps_qkv", bufs=2, space="PSUM") as ps_pv,
                tc.tile_pool(name="ps_tr", bufs=2, space="PSUM") as ps_tp1,
            ):
                phase1_tile(0, ps_pv, ps_tp1)
                # bulk rope-table load: scalar queue, split so early tiles
                # aren't gated behind the whole table
                nc.scalar.dma_start(out=w1_sb[:, 1:4, :], in_=w1[:, 1:4, :])
                nc.scalar.dma_start(out=w2_sb[:, 1:4, :], in_=w2[:, 1:4, :])
                nc.scalar.dma_start(out=w1_sb[:, 4:NT, :], in_=w1[:, 4:NT, :])
                nc.scalar.dma_start(out=w2_sb[:, 4:NT, :], in_=w2[:, 4:NT, :])
                nc.scalar.dma_start(out=wo_sb, in_=wo[:, :, :])
                for t in range(1, NT):
                    phase1_tile(t, ps_pv, ps_tp1)
            with (
                tc.tile_pool(name="ps_s", bufs=2, space="PSUM") as ps_qs,
                tc.tile_pool(name="ps_o", bufs=1, space="PSUM") as ps_po,
                tc.tile_pool(name="ps_y", bufs=2, space="PSUM") as ps_py,
                tc.tile_pool(name="ps_t2", bufs=2, space="PSUM") as ps_tp2,
            ):
                # group g's out-projection is spread one unit per chunk-step
                # through group g+1's attention to avoid head-of-line stalls
                units = iter(())
                for g in range(NQT):
                    oT_tiles = {}
                    attention_head(g, 0, ps_qs, ps_po, ps_tp2, oT_tiles, units)
                    attention_head(g, 1, ps_qs, ps_po, ps_tp2, oT_tiles, units)
                    for u in units:
                        u()
                    units = out_proj_units(g, ps_py, oT_tiles)
                for u in units:
                    u()
    nc.compile()
    return nc


_NC_CACHE = None


def _get_nc():
    global _NC_CACHE
    if _NC_CACHE is None:
        _NC_CACHE = build()
    return _NC_CACHE


def prep_inputs(x, w_qkv, w_out):
    """Host-side sharding/layout prep. Returns list of per-core input maps."""
    wnp = ml_dtypes.bfloat16 if WDTYPE == "bf16" else np.float32
    x2d = np.asarray(x, dtype=np.float32).reshape(L, D)
    w_qkv = np.asarray(w_qkv, dtype=np.float32)
    w_out = np.asarray(w_out, dtype=np.float32)

    # xt[t, c, p, l] = x2d[t*128 + l, c*128 + p]
    # [t, p(hid), c, l] so each per-tile DMA is one linear stream
    xt = np.ascontiguousarray(
        x2d.reshape(NT, LT, NHC, HC).transpose(0, 3, 2, 1)).astype(wnp)

    # rope coefficient tables
    inv_freq = 1.0 / (ROPE_BASE ** (np.arange(0, HD, 2, dtype=np.float64) / HD))
    pos = np.arange(L, dtype=np.float64)[:, None]
    ang = pos * inv_freq[None, :]                       # [L, 64]
    cos, sin = np.cos(ang), np.sin(ang)
    w1 = np.zeros((L, HD), dtype=np.float32)
    w2 = np.zeros((L, HD), dtype=np.float32)
    w1[:, 0::2] = -sin
    w1[:, 1::2] = cos
    w2[:, 0::2] = cos
    w2[:, 1::2] = sin
    w1 = np.ascontiguousarray(w1.reshape(NT, LT, HD).transpose(1, 0, 2))
    w2 = np.ascontiguousarray(w2.reshape(NT, LT, HD).transpose(1, 0, 2))

    # causal mask tile for diagonal blocks
    i = np.arange(128)[:, None]
    j = np.arange(128)[None, :]
    mask4 = np.where(i <= j, 0.0, NEG).astype(np.float32)  # [128, 128]
    ident = np.eye(128, dtype=np.float32).astype(wnp)

    in_maps = []
    for c in range(N_CORES):
        h0 = HPC * c
        rows = []
        for part in range(3):  # q, k, v
            for hh in range(HPC):
                base = part * D + (h0 + hh) * HD
                rows.append(w_qkv[base:base + HD])
        w_c = np.concatenate(rows, axis=0)              # [768, D]
        wt = np.ascontiguousarray(w_c.T).astype(wnp)    # [D, 768]
        wo = np.ascontiguousarray(
            w_out[:, h0 * HD:(h0 + HPC) * HD].T.reshape(HPC, HD, D)
            .transpose(1, 0, 2)).astype(wnp)
        in_maps.append({
            "xt": xt, "wt": wt, "wo": wo, "w1": w1, "w2": w2,
            "mask4": mask4, "ident": ident,
        })
    return in_maps


def kernel(x, w_qkv, w_out, mask, _trace=False):
    """Full MHA forward. Returns [1, L, D] float32."""
    nc = _get_nc()
    in_maps = prep_inputs(x, w_qkv, w_out)
    res = run_bass_kernel_spmd(nc, in_maps, core_ids=list(range(N_CORES)),
                               trace=_trace)
    acc = np.zeros((L, D), dtype=np.float32)
    for r in res.results:
        acc += r["out"]
    out = acc.reshape(1, L, D)
    if _trace:
        return out, res
    return out


# revision 17
# speedup vs baseline: 1.4303x; 1.0105x over previous
"""Trainium2 Bass kernel for 16-head causal MHA (RMSNorm+RoPE on q,k).

Tensor-parallel over heads: 8 cores x 2 heads each. Each core computes
qkv projection for its heads, norm+rope, causal attention, and a partial
out-projection; the host sums the 8 partial outputs.

v2 layout notes (vs the original v-stationary design):
- Scores are computed transposed [k, q]; exp tiles then serve as the
  STATIONARY matmul operand for PV, with a ones-column appended to V, so
  the PV output lands as [q, v|den] in PSUM: the softmax denominator is
  column 128 and the division becomes a per-partition ACT copy-scale.
  This removes all denominator matmuls and all per-column scaling ops.
- All 128x128 transposes (phase-1 q/k blocks, phase-2 o blocks) go
  through the DMA xbar transpose engine instead of TensorE.
- RoPE is computed into a de-interleaved [odd-half | even-half] d-order,
  identically for q and k (dot products unchanged); v / out_proj keep the
  natural d-order.
- exp() is computed without max-subtraction: post-RMSNorm |q.k|/sqrt(hd)
  <= sqrt(128), so exp is bounded by ~8.2e4. Masked (upper-triangular)
  score blocks are skipped entirely; diagonal blocks get an additive -1e9.
"""
import os
import ml_dtypes
import numpy as np

import concourse.bacc as bacc
import concourse.mybir as mybir
import concourse.tile as tile
from concourse.ap import AP
from concourse.bass_utils import run_bass_kernel_spmd


def _bcast_mid(ap2d, n):
    """[128, X] -> [128, n, X] with step-0 middle dim."""
    return AP(tensor=ap2d.tensor, offset=ap2d.offset,
              ap=[list(ap2d.ap[0]), [0, n], list(ap2d.ap[1])])

F32 = mybir.dt.float32
F32R = mybir.dt.float32r
BF16 = mybir.dt.bfloat16
WDTYPE = os.environ.get("MHA_WDTYPE", "bf16")
WDT = BF16 if WDTYPE == "bf16" else F32R
AF = mybir.ActivationFunctionType
ALU = mybir.AluOpType
AX = mybir.AxisListType

N_CORES = 8
L = 2048
D = 2048
HD = 128
N_HEAD = 16
HPC = N_HEAD // N_CORES  # heads per core = 2
LT = 128                 # L-tile rows
NT = L // LT             # 16 L-tiles
HC = 128                 # hid chunk
NHC = D // HC            # 16 hid chunks
QT = 512                 # q-tile width in attention
NQT = L // QT            # 4
VW = 130                 # v row stride (128 dims + ones col + pad)
EPS = 1e-5
ROPE_BASE = 10000.0
SCALE = 1.0 / float(np.sqrt(HD))
NEG = -1.0e9


def build():
    nc = bacc.Bacc("TRN2", target_bir_lowering=False, debug=False,
                   enable_asserts=False, num_devices=N_CORES)

    # Per-core external inputs (host-prepped layouts; see prep_inputs()).
    xt = nc.dram_tensor("xt", [NT, HC, NHC, LT], WDT, kind="ExternalInput")
    wt = nc.dram_tensor("wt", [D, 6 * HD], WDT, kind="ExternalInput")
    wo = nc.dram_tensor("wo", [HD, HPC, D], WDT, kind="ExternalInput")
    w1 = nc.dram_tensor("w1", [LT, NT, HD], F32, kind="ExternalInput")
    w2 = nc.dram_tensor("w2", [LT, NT, HD], F32, kind="ExternalInput")
    mask4 = nc.dram_tensor("mask4", [128, 128], F32, kind="ExternalInput")
    ident_in = nc.dram_tensor("ident", [128, 128], WDT, kind="ExternalInput")

    out = nc.dram_tensor("out", [L, D], F32, kind="ExternalOutput")

    with tile.TileContext(nc) as tc:
        with (
            tc.tile_pool(name="const", bufs=1) as constp,
            tc.tile_pool(name="wpool", bufs=1) as wpool,
            tc.tile_pool(name="persist", bufs=1) as persist,
            tc.tile_pool(name="xin", bufs=3) as xin,
            tc.tile_pool(name="qkv", bufs=3) as qkvp,
            tc.tile_pool(name="attn", bufs=4) as attnp,
            tc.tile_pool(name="res", bufs=4) as resp,
        ):
            # ---- weights resident (w chunks first: they gate the GEMMs).
            # Split across the two HWDGE queues for startup bandwidth.
            w_sb = wpool.tile([128, NHC, 6 * HD], WDT)
            for c in range(NHC):
                eng = nc.sync if c % 2 == 0 else nc.scalar
                eng.dma_start(out=w_sb[:, c, :],
                              in_=wt[c * 128:(c + 1) * 128, :])
            w1_sb = constp.tile([128, NT, HD], F32)
            nc.gpsimd.dma_start(out=w1_sb[:, 0, :], in_=w1[:, 0, :])
            w2_sb = constp.tile([128, NT, HD], F32)
            nc.gpsimd.dma_start(out=w2_sb[:, 0, :], in_=w2[:, 0, :])
            mask_sb = constp.tile([128, 128], F32)
            nc.gpsimd.dma_start(out=mask_sb, in_=mask4[:, :])
            ident = constp.tile([128, 128], WDT)
            nc.gpsimd.dma_start(out=ident, in_=ident_in[:, :])
            wo_sb = wpool.tile([128, HPC, D], WDT)
            eps_sb = constp.tile([128, 1], F32)
            nc.vector.memset(eps_sb, EPS)

            # persistent activations
            # v_sb: [kpos-part, t, head, 130]; col 128 is the ones column
            # feeding the softmax denominator, col 129 is alignment pad.
            # Fill with ones up front; phase-1 copies overwrite cols 0:128.
            v_sb = persist.tile([128, NT, HPC, VW], WDT)
            nc.vector.memset(v_sb, 1.0)
            qT = persist.tile([128, HPC, L], WDT)               # [d, h, L]
            kT = persist.tile([128, HPC, L], WDT)

            def load_x(t):
                x_tile = xin.tile([128, NHC, LT], WDT, tag="x", name="x_tile")
                nc.scalar.dma_start(out=x_tile, in_=xt[t, :, :, :])
                return x_tile

            def phase1_tile(t, x_tile, ps_pv, ps_tp):

                p_qk = ps_pv.tile([128, 4 * HD], F32, tag="pqk", name="p_qk")
                p_v = ps_pv.tile([128, HPC * HD], F32, tag="pv", name="p_v")
                for c in range(NHC):
                    nc.tensor.matmul(p_qk, x_tile[:, c, :], w_sb[:, c, 0:4 * HD],
                                     start=(c == 0), stop=(c == NHC - 1))
                    nc.tensor.matmul(p_v, x_tile[:, c, :],
                                     w_sb[:, c, 4 * HD:6 * HD],
                                     start=(c == 0), stop=(c == NHC - 1))

                nc.scalar.copy(
                    v_sb[:, t, :, 0:HD],
                    p_v.rearrange("p (h d) -> p h d", h=HPC))

                # rms-norm scale: s = 1/sqrt(mean(x^2) + eps) per (L, seg)
                sq = qkvp.tile([128, 4 * HD], F32, tag="sq", name="sq")
                nc.scalar.activation(sq, p_qk, AF.Square)
                ssum = qkvp.tile([128, 4], F32, tag="ssum", name="ssum")
                nc.vector.reduce_sum(ssum, sq.rearrange("p (g d) -> p g d", g=4),
                                     axis=AX.X)
                nc.scalar.activation(ssum, ssum, AF.Sqrt, scale=1.0 / HD,
                                     bias=eps_sb)
                s_val = qkvp.tile([128, 4], F32, tag="sval", name="s_val")
                nc.vector.reciprocal(s_val, ssum)

                # rope (batched): qk_n = qk * s; z = qk_n .* W; pair-add
                qk_n = qkvp.tile([128, 4 * HD], F32, tag="qkn", name="qk_n")
                nc.vector.tensor_mul(qk_n.rearrange("p (g d) -> p g d", g=4),
                                     p_qk.rearrange("p (g d) -> p g d", g=4),
                                     s_val.to_broadcast([128, 4, HD]))
                roped = qkvp.tile([128, 4 * HD], WDT, tag="roped", name="roped")
                roped4 = roped.rearrange("p (g h x) -> p g h x", g=4, h=2)
                for half, wtab in ((0, w1_sb), (1, w2_sb)):
                    z = qkvp.tile([128, 4 * HD], F32, tag="z", name="z")
                    nc.vector.tensor_mul(z.rearrange("p (g d) -> p g d", g=4),
                                         qk_n.rearrange("p (g d) -> p g d", g=4),
                                         _bcast_mid(wtab[:, t, :], 4))
                    with nc.allow_low_precision("2-elem rope pairs"):
                        nc.vector.reduce_sum(
                            roped4[:, :, half, :],
                            z.rearrange("p (g x two) -> p g x two", g=4, two=2),
                            axis=AX.X)

                # q blocks transpose on PE; k blocks (latency-tolerant: only
                # needed in phase 2) go through the xbar DMA transpose engine
                for seg in range(2):
                    p_tr = ps_tp.tile([128, 128], WDT, tag="tp", name="p_tr")
                    nc.tensor.transpose(
                        p_tr, roped[:, seg * HD:(seg + 1) * HD], ident)
                    nc.scalar.copy(qT[:, seg, t * LT:(t + 1) * LT], p_tr)
                for seg in range(2, 4):
                    nc.sync.dma_start(
                        out=kT[:, seg - 2, t * LT:(t + 1) * LT],
                        in_=roped[:, seg * HD:(seg + 1) * HD],
                        transpose=True)

            # p_o packs 4 q-chunks of [v(128)|den(1)] into 2 PSUM banks.
            # qc0/qc3's start=True clears their bank's has_written bits; qc1/2
            # then overwrite-on-clear-bit at kc==0 (PE FIFO runs qc0 first).
            PO_OFF = (0, 130, 260, 512)

            def attention_head(g, h, ps_qs, ps_po, ps_tp2, oT_tiles, filler):
                """Scores + exp + PV for one (q-group, head)."""
                nkc = 4 * g + 4
                p_o = ps_po.tile([128, 1024], F32, tag="po", name="p_o")
                exps = {}

                def score(kc):
                    diag = kc >= 4 * g
                    q0 = (kc - 4 * g) * 128 if diag else 0
                    p_s = ps_qs.tile([128, QT], F32, tag="qs", name="p_s")
                    nc.tensor.matmul(
                        p_s[:, q0:QT], kT[:, h, kc * 128:(kc + 1) * 128],
                        qT[:, h, g * QT + q0:(g + 1) * QT],
                        start=True, stop=True)
                    if diag:
                        nc.vector.tensor_add(
                            p_s[:, q0:q0 + 128], p_s[:, q0:q0 + 128], mask_sb)
                    expT = attnp.tile([128, QT], WDT, tag="expT", bufs=6,
                                      name="expT")
                    nc.scalar.activation(expT[:, q0:QT], p_s[:, q0:QT],
                                         AF.Exp, scale=SCALE)
                    exps[kc] = expT

                def pv(kc):
                    expT = exps.pop(kc)
                    for qc in range(max(0, kc - 4 * g), 4):
                        off = PO_OFF[qc]
                        nc.tensor.matmul(
                            p_o[:, off:off + 129],
                            expT[:, qc * 128:(qc + 1) * 128],
                            v_sb[:, kc, h, 0:129],
                            start=(kc == 0 and qc in (0, 3)),
                            stop=(kc == 4 * g + qc))

                def unit():
                    u = next(filler, None)
                    if u is not None:
                        u()

                # keep the score matmul one chunk ahead of PV on the PE queue;
                # interleave one out-proj unit of the previous group per step
                score(0)
                for kc in range(1, nkc):
                    score(kc)
                    pv(kc - 1)
                    unit()
                pv(nkc - 1)
                unit()

                # o_sb[q, vd] = p_o[q, 0:128] / den (den = col 128), then
                # transpose back to [vd, q] on PE for the out-projection
                for qc in range(4):
                    off = PO_OFF[qc]
                    inv = attnp.tile([128, 1], F32, tag="inv", bufs=4,
                                     name="inv")
                    nc.vector.reciprocal(inv, p_o[:, off + 128:off + 129])
                    o_sb = attnp.tile([128, HD], WDT, tag="osb", bufs=8,
                                      name="o_sb")
                    nc.scalar.activation(o_sb, p_o[:, off:off + HD], AF.Copy,
                                         scale=inv)
                    p_tr = ps_tp2.tile([128, 128], WDT, tag="tp2", name="p_tr2")
                    nc.tensor.transpose(p_tr, o_sb, ident)
                    oT = resp.tile([128, 128], WDT, tag="oT", bufs=16,
                                   name="oT")
                    nc.scalar.copy(oT, p_tr)
                    oT_tiles[(h, qc)] = oT

            def out_proj_units(g, ps_py, oT_tiles):
                for tt in range(4):
                    t = 4 * g + tt
                    for ec in range(4):
                        def u(t=t, tt=tt, ec=ec):
                            p_y = ps_py.tile([128, QT], F32, tag="py",
                                             name="p_y")
                            nc.tensor.matmul(
                                p_y, oT_tiles[(0, tt)],
                                wo_sb[:, 0, ec * 512:(ec + 1) * 512],
                                start=True, stop=False)
                            nc.tensor.matmul(
                                p_y, oT_tiles[(1, tt)],
                                wo_sb[:, 1, ec * 512:(ec + 1) * 512],
                                start=False, stop=True)
                            y = resp.tile([128, QT], F32, tag="y", bufs=4,
                                          name="y")
                            nc.vector.tensor_copy(y, p_y)
                            nc.gpsimd.dma_start(
                                out=out[t * LT:(t + 1) * LT,
                                        ec * 512:(ec + 1) * 512],
                                in_=y)
                        yield u

            with (
                tc.tile_pool(name="ps_qkv", bufs=2, space="PSUM") as ps_pv,
                tc.tile_pool(name="ps_tr", bufs=2, space="PSUM") as ps_tp1,
            ):
                # scalar HWDGE queue carries x tiles, interleaved with the
                # rope tables so neither gates the per-tile pipeline
                x0 = load_x(0)
                nc.scalar.dma_start(out=w1_sb[:, 1:6, :], in_=w1[:, 1:6, :])
                nc.scalar.dma_start(out=w2_sb[:, 1:6, :], in_=w2[:, 1:6, :])
                x1 = load_x(1)
                x2 = load_x(2)
                nc.scalar.dma_start(out=w1_sb[:, 6:NT, :], in_=w1[:, 6:NT, :])
                nc.scalar.dma_start(out=w2_sb[:, 6:NT, :], in_=w2[:, 6:NT, :])
                xs = {0: x0, 1: x1, 2: x2}
                phase1_tile(0, xs.pop(0), ps_pv, ps_tp1)
                for t in range(1, NT):
                    if t + 2 < NT:
                        xs[t + 2] = load_x(t + 2)
                    if t == 1:
                        nc.scalar.dma_start(out=wo_sb, in_=wo[:, :, :])
                    phase1_tile(t, xs.pop(t), ps_pv, ps_tp1)
            with (
                tc.tile_pool(name="ps_s", bufs=2, space="PSUM") as ps_qs,
                tc.tile_pool(name="ps_o", bufs=1, space="PSUM") as ps_po,
                tc.tile_pool(name="ps_y", bufs=2, space="PSUM") as ps_py,
                tc.tile_pool(name="ps_t2", bufs=2, space="PSUM") as ps_tp2,
            ):
                # group g's out-projection is spread one unit per chunk-step
                # through group g+1's attention to avoid head-of-line stalls
                units = iter(())
                for g in range(NQT):
                    oT_tiles = {}
                    attention_head(g, 0, ps_qs, ps_po, ps_tp2, oT_tiles, units)
                    attention_head(g, 1, ps_qs, ps_po, ps_tp2, oT_tiles, units)
                    for u in units:
                        u()
                    units = out_proj_units(g, ps_py, oT_tiles)
                for u in units:
                    u()
    nc.compile()
    return nc


_NC_CACHE = None


def _get_nc():
    global _NC_CACHE
    if _NC_CACHE is None:
        _NC_CACHE = build()
    return _NC_CACHE


def prep_inputs(x, w_qkv, w_out):
    """Host-side sharding/layout prep. Returns list of per-core input maps."""
    wnp = ml_dtypes.bfloat16 if WDTYPE == "bf16" else np.float32
    x2d = np.asarray(x, dtype=np.float32).reshape(L, D)
    w_qkv = np.asarray(w_qkv, dtype=np.float32)
    w_out = np.asarray(w_out, dtype=np.float32)

    # xt[t, c, p, l] = x2d[t*128 + l, c*128 + p]
    # [t, p(hid), c, l] so each per-tile DMA is one linear stream
    xt = np.ascontiguousarray(
        x2d.reshape(NT, LT, NHC, HC).transpose(0, 3, 2, 1)).astype(wnp)

    # rope coefficient tables
    inv_freq = 1.0 / (ROPE_BASE ** (np.arange(0, HD, 2, dtype=np.float64) / HD))
    pos = np.arange(L, dtype=np.float64)[:, None]
    ang = pos * inv_freq[None, :]                       # [L, 64]
    cos, sin = np.cos(ang), np.sin(ang)
    w1 = np.zeros((L, HD), dtype=np.float32)
    w2 = np.zeros((L, HD), dtype=np.float32)
    w1[:, 0::2] = -sin
    w1[:, 1::2] = cos
    w2[:, 0::2] = cos
    w2[:, 1::2] = sin
    w1 = np.ascontiguousarray(w1.reshape(NT, LT, HD).transpose(1, 0, 2))
    w2 = np.ascontiguousarray(w2.reshape(NT, LT, HD).transpose(1, 0, 2))

    # causal mask tile for diagonal blocks
    i = np.arange(128)[:, None]
    j = np.arange(128)[None, :]
    mask4 = np.where(i <= j, 0.0, NEG).astype(np.float32)  # [128, 128]
    ident = np.eye(128, dtype=np.float32).astype(wnp)

    in_maps = []
    for c in range(N_CORES):
        h0 = HPC * c
        rows = []
        for part in range(3):  # q, k, v
            for hh in range(HPC):
                base = part * D + (h0 + hh) * HD
                rows.append(w_qkv[base:base + HD])
        w_c = np.concatenate(rows, axis=0)              # [768, D]
        wt = np.ascontiguousarray(w_c.T).astype(wnp)    # [D, 768]
        wo = np.ascontiguousarray(
            w_out[:, h0 * HD:(h0 + HPC) * HD].T.reshape(HPC, HD, D)
            .transpose(1, 0, 2)).astype(wnp)
        in_maps.append({
            "xt": xt, "wt": wt, "wo": wo, "w1": w1, "w2": w2,
            "mask4": mask4, "ident": ident,
        })
    return in_maps


def kernel(x, w_qkv, w_out, mask, _trace=False):
    """Full MHA forward. Returns [1, L, D] float32."""
    nc = _get_nc()
    in_maps = prep_inputs(x, w_qkv, w_out)
    res = run_bass_kernel_spmd(nc, in_maps, core_ids=list(range(N_CORES)),
                               trace=_trace)
    acc = np.zeros((L, D), dtype=np.float32)
    for r in res.results:
        acc += r["out"]
    out = acc.reshape(1, L, D)
    if _trace:
        return out, res
    return out


# revision 19
# speedup vs baseline: 1.4591x; 1.0202x over previous
"""Trainium2 Bass kernel for 16-head causal MHA (RMSNorm+RoPE on q,k).

Tensor-parallel over heads: 8 cores x 2 heads each. Each core computes
qkv projection for its heads, norm+rope, causal attention, and a partial
out-projection; the host sums the 8 partial outputs.

v2 layout notes (vs the original v-stationary design):
- Scores are computed transposed [k, q]; exp tiles then serve as the
  STATIONARY matmul operand for PV, with a ones-column appended to V, so
  the PV output lands as [q, v|den] in PSUM: the softmax denominator is
  column 128 and the division becomes a per-partition ACT copy-scale.
  This removes all denominator matmuls and all per-column scaling ops.
- All 128x128 transposes (phase-1 q/k blocks, phase-2 o blocks) go
  through the DMA xbar transpose engine instead of TensorE.
- RoPE is computed into a de-interleaved [odd-half | even-half] d-order,
  identically for q and k (dot products unchanged); v / out_proj keep the
  natural d-order.
- exp() is computed without max-subtraction: post-RMSNorm |q.k|/sqrt(hd)
  <= sqrt(128), so exp is bounded by ~8.2e4. Masked (upper-triangular)
  score blocks are skipped entirely; diagonal blocks get an additive -1e9.
"""
import os
import ml_dtypes
import numpy as np

import concourse.bacc as bacc
import concourse.mybir as mybir
import concourse.tile as tile
from concourse.ap import AP
from concourse.bass_utils import run_bass_kernel_spmd


def _bcast_mid(ap2d, n):
    """[128, X] -> [128, n, X] with step-0 middle dim."""
    return AP(tensor=ap2d.tensor, offset=ap2d.offset,
              ap=[list(ap2d.ap[0]), [0, n], list(ap2d.ap[1])])

F32 = mybir.dt.float32
F32R = mybir.dt.float32r
BF16 = mybir.dt.bfloat16
WDTYPE = os.environ.get("MHA_WDTYPE", "bf16")
WDT = BF16 if WDTYPE == "bf16" else F32R
AF = mybir.ActivationFunctionType
ALU = mybir.AluOpType
AX = mybir.AxisListType

N_CORES = 8
L = 2048
D = 2048
HD = 128
N_HEAD = 16
HPC = N_HEAD // N_CORES  # heads per core = 2
LT = 128                 # L-tile rows
NT = L // LT             # 16 L-tiles
HC = 128                 # hid chunk
NHC = D // HC            # 16 hid chunks
QT = 512                 # q-tile width in attention
NQT = L // QT            # 4
VW = 130                 # v row stride (128 dims + ones col + pad)
EPS = 1e-5
ROPE_BASE = 10000.0
SCALE = 1.0 / float(np.sqrt(HD))
NEG = -1.0e9


def build():
    nc = bacc.Bacc("TRN2", target_bir_lowering=False, debug=False,
                   enable_asserts=False, num_devices=N_CORES)

    # Per-core external inputs (host-prepped layouts; see prep_inputs()).
    xt = nc.dram_tensor("xt", [NT, HC, NHC, LT], WDT, kind="ExternalInput")
    wt = nc.dram_tensor("wt", [D, 6 * HD], WDT, kind="ExternalInput")
    wo = nc.dram_tensor("wo", [HD, HPC, D], WDT, kind="ExternalInput")
    w1 = nc.dram_tensor("w1", [LT, NT, HD], F32, kind="ExternalInput")
    w2 = nc.dram_tensor("w2", [LT, NT, HD], F32, kind="ExternalInput")
    mask4 = nc.dram_tensor("mask4", [128, 128], F32, kind="ExternalInput")
    ident_in = nc.dram_tensor("ident", [128, 128], WDT, kind="ExternalInput")

    out = nc.dram_tensor("out", [L, D], F32, kind="ExternalOutput")

    with tile.TileContext(nc) as tc:
        with (
            tc.tile_pool(name="const", bufs=1) as constp,
            tc.tile_pool(name="wpool", bufs=1) as wpool,
            tc.tile_pool(name="persist", bufs=1) as persist,
            tc.tile_pool(name="xin", bufs=5) as xin,
            tc.tile_pool(name="qkv", bufs=3) as qkvp,
            tc.tile_pool(name="attn", bufs=4) as attnp,
            tc.tile_pool(name="res", bufs=4) as resp,
        ):
            # ---- weights resident (w chunks first: they gate the GEMMs).
            # Split across the two HWDGE queues for startup bandwidth.
            w_sb = wpool.tile([128, NHC, 6 * HD], WDT)
            for c in range(NHC):
                eng = nc.sync if c % 2 == 0 else nc.scalar
                eng.dma_start(out=w_sb[:, c, :],
                              in_=wt[c * 128:(c + 1) * 128, :])
            w1_sb = constp.tile([128, NT, HD], F32)
            nc.gpsimd.dma_start(out=w1_sb[:, 0, :], in_=w1[:, 0, :])
            w2_sb = constp.tile([128, NT, HD], F32)
            nc.gpsimd.dma_start(out=w2_sb[:, 0, :], in_=w2[:, 0, :])
            mask_sb = constp.tile([128, 128], F32)
            nc.gpsimd.dma_start(out=mask_sb, in_=mask4[:, :])
            ident = constp.tile([128, 128], WDT)
            nc.gpsimd.dma_start(out=ident, in_=ident_in[:, :])
            wo_sb = wpool.tile([128, HPC, D], WDT)
            eps_sb = constp.tile([128, 1], F32)
            nc.vector.memset(eps_sb, EPS)

            # persistent activations
            # v_sb: [kpos-part, t, head, 130]; col 128 is the ones column
            # feeding the softmax denominator, col 129 is alignment pad.
            # Fill with ones up front; phase-1 copies overwrite cols 0:128.
            v_sb = persist.tile([128, NT, HPC, VW], WDT)
            nc.vector.memset(v_sb, 1.0)
            qT = persist.tile([128, HPC, L], WDT)               # [d, h, L]
            kT = persist.tile([128, HPC, L], WDT)

            def load_x(t):
                x_tile = xin.tile([128, NHC, LT], WDT, tag="x", name="x_tile")
                nc.scalar.dma_start(out=x_tile, in_=xt[t, :, :, :])
                return x_tile

            def phase1_tile(t, x_tile, ps_pv, ps_tp):

                p_qk = ps_pv.tile([128, 4 * HD], F32, tag="pqk", name="p_qk")
                p_v = ps_pv.tile([128, HPC * HD], F32, tag="pv", name="p_v")
                for c in range(NHC):
                    nc.tensor.matmul(p_qk, x_tile[:, c, :], w_sb[:, c, 0:4 * HD],
                                     start=(c == 0), stop=(c == NHC - 1))
                    nc.tensor.matmul(p_v, x_tile[:, c, :],
                                     w_sb[:, c, 4 * HD:6 * HD],
                                     start=(c == 0), stop=(c == NHC - 1))

                nc.scalar.copy(
                    v_sb[:, t, :, 0:HD],
                    p_v.rearrange("p (h d) -> p h d", h=HPC))

                # rms-norm scale: s = 1/sqrt(mean(x^2) + eps) per (L, seg)
                sq = qkvp.tile([128, 4 * HD], F32, tag="sq", name="sq")
                nc.scalar.activation(sq, p_qk, AF.Square)
                ssum = qkvp.tile([128, 4], F32, tag="ssum", name="ssum")
                nc.vector.reduce_sum(ssum, sq.rearrange("p (g d) -> p g d", g=4),
                                     axis=AX.X)
                nc.scalar.activation(ssum, ssum, AF.Sqrt, scale=1.0 / HD,
                                     bias=eps_sb)
                s_val = qkvp.tile([128, 4], F32, tag="sval", name="s_val")
                nc.vector.reciprocal(s_val, ssum)

                # rope (batched): qk_n = qk * s; z = qk_n .* W; pair-add
                qk_n = qkvp.tile([128, 4 * HD], F32, tag="qkn", name="qk_n")
                nc.vector.tensor_mul(qk_n.rearrange("p (g d) -> p g d", g=4),
                                     p_qk.rearrange("p (g d) -> p g d", g=4),
                                     s_val.to_broadcast([128, 4, HD]))
                roped = qkvp.tile([128, 4 * HD], WDT, tag="roped", name="roped")
                roped4 = roped.rearrange("p (g h x) -> p g h x", g=4, h=2)
                for half, wtab in ((0, w1_sb), (1, w2_sb)):
                    z = qkvp.tile([128, 4 * HD], F32, tag="z", name="z")
                    nc.vector.tensor_mul(z.rearrange("p (g d) -> p g d", g=4),
                                         qk_n.rearrange("p (g d) -> p g d", g=4),
                                         _bcast_mid(wtab[:, t, :], 4))
                    with nc.allow_low_precision("2-elem rope pairs"):
                        nc.vector.reduce_sum(
                            roped4[:, :, half, :],
                            z.rearrange("p (g x two) -> p g x two", g=4, two=2),
                            axis=AX.X)

                # q blocks transpose on PE; k blocks (latency-tolerant: only
                # needed in phase 2) go through the xbar DMA transpose engine
                for seg in range(2):
                    p_tr = ps_tp.tile([128, 128], WDT, tag="tp", name="p_tr")
                    nc.tensor.transpose(
                        p_tr, roped[:, seg * HD:(seg + 1) * HD], ident)
                    nc.scalar.copy(qT[:, seg, t * LT:(t + 1) * LT], p_tr)
                for seg in range(2, 4):
                    nc.sync.dma_start(
                        out=kT[:, seg - 2, t * LT:(t + 1) * LT],
                        in_=roped[:, seg * HD:(seg + 1) * HD],
                        transpose=True)

            # p_o packs 4 q-chunks of [v(128)|den(1)] into 2 PSUM banks.
            # qc0/qc3's start=True clears their bank's has_written bits; qc1/2
            # then overwrite-on-clear-bit at kc==0 (PE FIFO runs qc0 first).
            PO_OFF = (0, 130, 260, 512)

            def attention_head(g, h, ps_qs, ps_po, ps_tp2, oT_tiles, filler):
                """Scores + exp + PV for one (q-group, head)."""
                nkc = 4 * g + 4
                p_o = ps_po.tile([128, 1024], F32, tag="po", name="p_o")
                exps = {}

                def score(kc):
                    diag = kc >= 4 * g
                    q0 = (kc - 4 * g) * 128 if diag else 0
                    p_s = ps_qs.tile([128, QT], F32, tag="qs", name="p_s")
                    nc.tensor.matmul(
                        p_s[:, q0:QT], kT[:, h, kc * 128:(kc + 1) * 128],
                        qT[:, h, g * QT + q0:(g + 1) * QT],
                        start=True, stop=True)
                    if diag:
                        nc.vector.tensor_add(
                            p_s[:, q0:q0 + 128], p_s[:, q0:q0 + 128], mask_sb)
                    expT = attnp.tile([128, QT], WDT, tag="expT", bufs=6,
                                      name="expT")
                    nc.scalar.activation(expT[:, q0:QT], p_s[:, q0:QT],
                                         AF.Exp, scale=SCALE)
                    exps[kc] = expT

                def pv(kc):
                    expT = exps.pop(kc)
                    for qc in range(max(0, kc - 4 * g), 4):
                        off = PO_OFF[qc]
                        nc.tensor.matmul(
                            p_o[:, off:off + 129],
                            expT[:, qc * 128:(qc + 1) * 128],
                            v_sb[:, kc, h, 0:129],
                            start=(kc == 0 and qc in (0, 3)),
                            stop=(kc == 4 * g + qc))

                def unit():
                    u = next(filler, None)
                    if u is not None:
                        u()

                # keep the score matmul one chunk ahead of PV on the PE queue;
                # interleave one out-proj unit of the previous group per step
                score(0)
                for kc in range(1, nkc):
                    score(kc)
                    pv(kc - 1)
                    unit()
                pv(nkc - 1)
                unit()

                # o_sb[q, vd] = p_o[q, 0:128] / den (den = col 128), then
                # transpose back to [vd, q] on PE for the out-projection
                for qc in range(4):
                    off = PO_OFF[qc]
                    inv = attnp.tile([128, 1], F32, tag="inv", bufs=4,
                                     name="inv")
                    nc.vector.reciprocal(inv, p_o[:, off + 128:off + 129])
                    o_sb = attnp.tile([128, HD], WDT, tag="osb", bufs=8,
                                      name="o_sb")
                    nc.scalar.activation(o_sb, p_o[:, off:off + HD], AF.Copy,
                                         scale=inv)
                    p_tr = ps_tp2.tile([128, 128], WDT, tag="tp2", name="p_tr2")
                    nc.tensor.transpose(p_tr, o_sb, ident)
                    oT = resp.tile([128, 128], WDT, tag="oT", bufs=16,
                                   name="oT")
                    nc.scalar.copy(oT, p_tr)
                    oT_tiles[(h, qc)] = oT

            def out_proj_units(g, ps_py, oT_tiles):
                for tt in range(4):
                    t = 4 * g + tt
                    for ec in range(4):
                        def u(t=t, tt=tt, ec=ec):
                            p_y = ps_py.tile([128, QT], F32, tag="py",
                                             name="p_y")
                            nc.tensor.matmul(
                                p_y, oT_tiles[(0, tt)],
                                wo_sb[:, 0, ec * 512:(ec + 1) * 512],
                                start=True, stop=False)
                            nc.tensor.matmul(
                                p_y, oT_tiles[(1, tt)],
                                wo_sb[:, 1, ec * 512:(ec + 1) * 512],
                                start=False, stop=True)
                            y = resp.tile([128, QT], F32, tag="y", bufs=4,
                                          name="y")
                            nc.vector.tensor_copy(y, p_y)
                            nc.gpsimd.dma_start(
                                out=out[t * LT:(t + 1) * LT,
                                        ec * 512:(ec + 1) * 512],
                                in_=y)
                        yield u

            with (
                tc.tile_pool(name="ps_qkv", bufs=2, space="PSUM") as ps_pv,
                tc.tile_pool(name="ps_tr", bufs=2, space="PSUM") as ps_tp1,
            ):
                # scalar HWDGE queue carries x tiles, interleaved with the
                # rope tables so neither gates the per-tile pipeline; x is
                # prefetched 4 tiles deep so the issue (which serializes
                # behind each tile's ACT chain) never gates the GEMM
                xs = {0: load_x(0)}
                nc.scalar.dma_start(out=w1_sb[:, 1:6, :], in_=w1[:, 1:6, :])
                nc.scalar.dma_start(out=w2_sb[:, 1:6, :], in_=w2[:, 1:6, :])
                xs[1] = load_x(1)
                xs[2] = load_x(2)
                nc.scalar.dma_start(out=w1_sb[:, 6:NT, :], in_=w1[:, 6:NT, :])
                nc.scalar.dma_start(out=w2_sb[:, 6:NT, :], in_=w2[:, 6:NT, :])
                xs[3] = load_x(3)
                xs[4] = load_x(4)
                phase1_tile(0, xs.pop(0), ps_pv, ps_tp1)
                for t in range(1, NT):
                    if t == 1:
                        nc.scalar.dma_start(out=wo_sb, in_=wo[:, :, :])
                    phase1_tile(t, xs.pop(t), ps_pv, ps_tp1)
                    if t + 4 < NT:
                        xs[t + 4] = load_x(t + 4)
            with (
                tc.tile_pool(name="ps_s", bufs=2, space="PSUM") as ps_qs,
                tc.tile_pool(name="ps_o", bufs=1, space="PSUM") as ps_po,
                tc.tile_pool(name="ps_y", bufs=2, space="PSUM") as ps_py,
                tc.tile_pool(name="ps_t2", bufs=2, space="PSUM") as ps_tp2,
            ):
                # group g's out-projection is spread one unit per chunk-step
                # through group g+1's attention to avoid head-of-line stalls
                units = iter(())
                for g in range(NQT):
                    oT_tiles = {}
                    attention_head(g, 0, ps_qs, ps_po, ps_tp2, oT_tiles, units)
                    attention_head(g, 1, ps_qs, ps_po, ps_tp2, oT_tiles, units)
                    for u in units:
                        u()
                    units = out_proj_units(g, ps_py, oT_tiles)
                for u in units:
                    u()
    nc.compile()
    return nc


_NC_CACHE = None


def _get_nc():
    global _NC_CACHE
    if _NC_CACHE is None:
        _NC_CACHE = build()
    return _NC_CACHE


def prep_inputs(x, w_qkv, w_out):
    """Host-side sharding/layout prep. Returns list of per-core input maps."""
    wnp = ml_dtypes.bfloat16 if WDTYPE == "bf16" else np.float32
    x2d = np.asarray(x, dtype=np.float32).reshape(L, D)
    w_qkv = np.asarray(w_qkv, dtype=np.float32)
    w_out = np.asarray(w_out, dtype=np.float32)

    # xt[t, c, p, l] = x2d[t*128 + l, c*128 + p]
    # [t, p(hid), c, l] so each per-tile DMA is one linear stream
    xt = np.ascontiguousarray(
        x2d.reshape(NT, LT, NHC, HC).transpose(0, 3, 2, 1)).astype(wnp)

    # rope coefficient tables
    inv_freq = 1.0 / (ROPE_BASE ** (np.arange(0, HD, 2, dtype=np.float64) / HD))
    pos = np.arange(L, dtype=np.float64)[:, None]
    ang = pos * inv_freq[None, :]                       # [L, 64]
    cos, sin = np.cos(ang), np.sin(ang)
    w1 = np.zeros((L, HD), dtype=np.float32)
    w2 = np.zeros((L, HD), dtype=np.float32)
    w1[:, 0::2] = -sin
    w1[:, 1::2] = cos
    w2[:, 0::2] = cos
    w2[:, 1::2] = sin
    w1 = np.ascontiguousarray(w1.reshape(NT, LT, HD).transpose(1, 0, 2))
    w2 = np.ascontiguousarray(w2.reshape(NT, LT, HD).transpose(1, 0, 2))

    # causal mask tile for diagonal blocks
    i = np.arange(128)[:, None]
    j = np.arange(128)[None, :]
    mask4 = np.where(i <= j, 0.0, NEG).astype(np.float32)  # [128, 128]
    ident = np.eye(128, dtype=np.float32).astype(wnp)

    in_maps = []
    for c in range(N_CORES):
        h0 = HPC * c
        rows = []
        for part in range(3):  # q, k, v
            for hh in range(HPC):
                base = part * D + (h0 + hh) * HD
                rows.append(w_qkv[base:base + HD])
        w_c = np.concatenate(rows, axis=0)              # [768, D]
        wt = np.ascontiguousarray(w_c.T).astype(wnp)    # [D, 768]
        wo = np.ascontiguousarray(
            w_out[:, h0 * HD:(h0 + HPC) * HD].T.reshape(HPC, HD, D)
            .transpose(1, 0, 2)).astype(wnp)
        in_maps.append({
            "xt": xt, "wt": wt, "wo": wo, "w1": w1, "w2": w2,
            "mask4": mask4, "ident": ident,
        })
    return in_maps


def kernel(x, w_qkv, w_out, mask, _trace=False):
    """Full MHA forward. Returns [1, L, D] float32."""
    nc = _get_nc()
    in_maps = prep_inputs(x, w_qkv, w_out)
    res = run_bass_kernel_spmd(nc, in_maps, core_ids=list(range(N_CORES)),
                               trace=_trace)
    acc = np.zeros((L, D), dtype=np.float32)
    for r in res.results:
        acc += r["out"]
    out = acc.reshape(1, L, D)
    if _trace:
        return out, res
    return out


# revision 24
# speedup vs baseline: 1.6625x; 1.1394x over previous
"""Trainium2 Bass kernel for 16-head causal MHA (RMSNorm+RoPE on q,k).

Tensor-parallel over heads: 8 cores x 2 heads each. Each core computes
qkv projection for its heads, norm+rope, causal attention, and a partial
out-projection; the host sums the 8 partial outputs.

v2 layout notes (vs the original v-stationary design):
- Scores are computed transposed [k, q]; exp tiles then serve as the
  STATIONARY matmul operand for PV, with a ones-column appended to V, so
  the PV output lands as [q, v|den] in PSUM: the softmax denominator is
  column 128 and the division becomes a per-partition ACT copy-scale.
  This removes all denominator matmuls and all per-column scaling ops.
- All 128x128 transposes (phase-1 q/k blocks, phase-2 o blocks) go
  through the DMA xbar transpose engine instead of TensorE.
- RoPE is computed into a de-interleaved [odd-half | even-half] d-order,
  identically for q and k (dot products unchanged); v / out_proj keep the
  natural d-order.
- exp() is computed without max-subtraction: post-RMSNorm |q.k|/sqrt(hd)
  <= sqrt(128), so exp is bounded by ~8.2e4. Masked (upper-triangular)
  score blocks are skipped entirely; diagonal blocks get an additive -1e9.
"""
import os
import ml_dtypes
import numpy as np

import concourse.bacc as bacc
import concourse.mybir as mybir
import concourse.tile as tile
from concourse.ap import AP
from concourse.bass_utils import run_bass_kernel_spmd


def _bcast_mid(ap2d, n):
    """[128, X] -> [128, n, X] with step-0 middle dim."""
    return AP(tensor=ap2d.tensor, offset=ap2d.offset,
              ap=[list(ap2d.ap[0]), [0, n], list(ap2d.ap[1])])

F32 = mybir.dt.float32
F32R = mybir.dt.float32r
BF16 = mybir.dt.bfloat16
WDTYPE = os.environ.get("MHA_WDTYPE", "bf16")
WDT = BF16 if WDTYPE == "bf16" else F32R
AF = mybir.ActivationFunctionType
ALU = mybir.AluOpType
AX = mybir.AxisListType

N_CORES = 8
L = 2048
D = 2048
HD = 128
N_HEAD = 16
HPC = N_HEAD // N_CORES  # heads per core = 2
LT = 128                 # L-tile rows
NT = L // LT             # 16 L-tiles
HC = 128                 # hid chunk
NHC = D // HC            # 16 hid chunks
QT = 512                 # q-tile width in attention
NQT = L // QT            # 4
VW = 130                 # v row stride (128 dims + ones col + pad)
EPS = 1e-5
ROPE_BASE = 10000.0
SCALE = 1.0 / float(np.sqrt(HD))
NEG = -1.0e9


def build():
    nc = bacc.Bacc("TRN2", target_bir_lowering=False, debug=False,
                   enable_asserts=False, num_devices=N_CORES)

    # Per-core external inputs (host-prepped layouts; see prep_inputs()).
    xt = nc.dram_tensor("xt", [NT, HC, NHC, LT], WDT, kind="ExternalInput")
    wt = nc.dram_tensor("wt", [D, 6 * HD], WDT, kind="ExternalInput")
    wo = nc.dram_tensor("wo", [HD, HPC, D], WDT, kind="ExternalInput")
    w1 = nc.dram_tensor("w1", [LT, NT, HD], F32, kind="ExternalInput")
    w2 = nc.dram_tensor("w2", [LT, NT, HD], F32, kind="ExternalInput")
    mask4 = nc.dram_tensor("mask4", [128, 128], F32, kind="ExternalInput")
    ident_in = nc.dram_tensor("ident", [128, 128], WDT, kind="ExternalInput")

    out = nc.dram_tensor("out", [L, D], F32, kind="ExternalOutput")

    with tile.TileContext(nc) as tc:
        with (
            tc.tile_pool(name="const", bufs=1) as constp,
            tc.tile_pool(name="wpool", bufs=1) as wpool,
            tc.tile_pool(name="persist", bufs=1) as persist,
            tc.tile_pool(name="xin", bufs=5) as xin,
            tc.tile_pool(name="qkv", bufs=3) as qkvp,
            tc.tile_pool(name="attn", bufs=4) as attnp,
            tc.tile_pool(name="res", bufs=4) as resp,
        ):
            # ---- weights resident (w chunks first: they gate the GEMMs).
            # Split across the two HWDGE queues for startup bandwidth.
            w_sb = wpool.tile([128, NHC, 6 * HD], WDT)
            for c in range(NHC):
                eng = nc.sync if c % 2 == 0 else nc.scalar
                eng.dma_start(out=w_sb[:, c, :],
                              in_=wt[c * 128:(c + 1) * 128, :])
            w1_sb = constp.tile([128, NT, HD], F32)
            nc.gpsimd.dma_start(out=w1_sb[:, 0, :], in_=w1[:, 0, :])
            w2_sb = constp.tile([128, NT, HD], F32)
            nc.gpsimd.dma_start(out=w2_sb[:, 0, :], in_=w2[:, 0, :])
            mask_sb = constp.tile([128, 128], F32)
            nc.gpsimd.dma_start(out=mask_sb, in_=mask4[:, :])
            ident = constp.tile([128, 128], WDT)
            nc.gpsimd.dma_start(out=ident, in_=ident_in[:, :])
            wo_sb = wpool.tile([128, HPC, D], WDT)
            eps_sb = constp.tile([128, 1], F32)
            nc.vector.memset(eps_sb, EPS)

            # persistent activations
            # v_sb: [kpos-part, t, head, 130]; col 128 is the ones column
            # feeding the softmax denominator, col 129 is alignment pad.
            # Fill with ones up front; phase-1 copies overwrite cols 0:128.
            v_sb = persist.tile([128, NT, HPC, VW], WDT)
            nc.vector.memset(v_sb, 1.0)
            qT = persist.tile([128, HPC, L], WDT)               # [d, h, L]
            kT = persist.tile([128, HPC, L], WDT)

            def load_x(t):
                x_tile = xin.tile([128, NHC, LT], WDT, tag="x", name="x_tile")
                nc.gpsimd.dma_start(out=x_tile, in_=xt[t, :, :, :])
                return x_tile

            def phase1_tile(t, x_tile, ps_pv, ps_tp):

                p_qk = ps_pv.tile([128, 4 * HD], F32, tag="pqk", name="p_qk")
                p_v = ps_pv.tile([128, HPC * HD], F32, tag="pv", name="p_v")
                for c in range(NHC):
                    nc.tensor.matmul(p_qk, x_tile[:, c, :], w_sb[:, c, 0:4 * HD],
                                     start=(c == 0), stop=(c == NHC - 1))
                    nc.tensor.matmul(p_v, x_tile[:, c, :],
                                     w_sb[:, c, 4 * HD:6 * HD],
                                     start=(c == 0), stop=(c == NHC - 1))

                nc.scalar.copy(
                    v_sb[:, t, :, 0:HD],
                    p_v.rearrange("p (h d) -> p h d", h=HPC))

                # rms-norm scale: s = 1/sqrt(mean(x^2) + eps) per (L, seg)
                sq = qkvp.tile([128, 4 * HD], F32, tag="sq", name="sq")
                nc.scalar.activation(sq, p_qk, AF.Square)
                ssum = qkvp.tile([128, 4], F32, tag="ssum", name="ssum")
                nc.vector.reduce_sum(ssum, sq.rearrange("p (g d) -> p g d", g=4),
                                     axis=AX.X)
                nc.scalar.activation(ssum, ssum, AF.Sqrt, scale=1.0 / HD,
                                     bias=eps_sb)
                s_val = qkvp.tile([128, 4], F32, tag="sval", name="s_val")
                nc.vector.reciprocal(s_val, ssum)

                # rope (batched): qk_n = qk * s; z = qk_n .* W; pair-add
                qk_n = qkvp.tile([128, 4 * HD], F32, tag="qkn", name="qk_n")
                nc.vector.tensor_mul(qk_n.rearrange("p (g d) -> p g d", g=4),
                                     p_qk.rearrange("p (g d) -> p g d", g=4),
                                     s_val.to_broadcast([128, 4, HD]))
                roped = qkvp.tile([128, 4 * HD], WDT, tag="roped", name="roped")
                roped4 = roped.rearrange("p (g h x) -> p g h x", g=4, h=2)
                for half, wtab in ((0, w1_sb), (1, w2_sb)):
                    z = qkvp.tile([128, 4 * HD], F32, tag="z", name="z")
                    nc.vector.tensor_mul(z.rearrange("p (g d) -> p g d", g=4),
                                         qk_n.rearrange("p (g d) -> p g d", g=4),
                                         _bcast_mid(wtab[:, t, :], 4))
                    with nc.allow_low_precision("2-elem rope pairs"):
                        nc.vector.reduce_sum(
                            roped4[:, :, half, :],
                            z.rearrange("p (g x two) -> p g x two", g=4, two=2),
                            axis=AX.X)

                # transpose the 4 roped [128,128] blocks into qT/kT on PE
                for seg in range(4):
                    tgt = qT if seg < 2 else kT
                    h = seg % 2
                    p_tr = ps_tp.tile([128, 128], WDT, tag="tp", name="p_tr")
                    nc.tensor.transpose(
                        p_tr, roped[:, seg * HD:(seg + 1) * HD], ident)
                    nc.scalar.copy(tgt[:, h, t * LT:(t + 1) * LT], p_tr)

            # p_o packs 4 q-chunks of [v(128)|den(1)] into 2 PSUM banks.
            # qc0/qc3's start=True clears their bank's has_written bits; qc1/2
            # then overwrite-on-clear-bit at kc==0 (PE FIFO runs qc0 first).
            PO_OFF = (0, 130, 260, 512)

            def attention_head(g, h, ps_qs, ps_po, ps_tp2, oT_tiles, filler):
                """Scores + exp + PV for one (q-group, head)."""
                nkc = 4 * g + 4
                p_o = ps_po.tile([128, 1024], F32, tag="po", name="p_o")
                exps = {}

                def score(kc):
                    diag = kc >= 4 * g
                    q0 = (kc - 4 * g) * 128 if diag else 0
                    p_s = ps_qs.tile([128, QT], F32, tag="qs", name="p_s")
                    nc.tensor.matmul(
                        p_s[:, q0:QT], kT[:, h, kc * 128:(kc + 1) * 128],
                        qT[:, h, g * QT + q0:(g + 1) * QT],
                        start=True, stop=True)
                    if diag:
                        nc.vector.tensor_add(
                            p_s[:, q0:q0 + 128], p_s[:, q0:q0 + 128], mask_sb)
                    expT = attnp.tile([128, QT], WDT, tag="expT", bufs=6,
                                      name="expT")
                    nc.scalar.activation(expT[:, q0:QT], p_s[:, q0:QT],
                                         AF.Exp, scale=SCALE)
                    exps[kc] = expT

                def pv(kc):
                    expT = exps.pop(kc)
                    for qc in range(max(0, kc - 4 * g), 4):
                        off = PO_OFF[qc]
                        nc.tensor.matmul(
                            p_o[:, off:off + 129],
                            expT[:, qc * 128:(qc + 1) * 128],
                            v_sb[:, kc, h, 0:129],
                            start=(kc == 0 and qc in (0, 3)),
                            stop=(kc == 4 * g + qc))

                def unit():
                    u = next(filler, None)
                    if u is not None:
                        u()

                # keep the score matmul one chunk ahead of PV on the PE queue;
                # interleave one out-proj unit of the previous group per step
                score(0)
                for kc in range(1, nkc):
                    score(kc)
                    pv(kc - 1)
                    unit()
                pv(nkc - 1)
                unit()

                # o_sb[q, vd] = p_o[q, 0:128] / den (den = col 128), then
                # transpose back to [vd, q] on PE for the out-projection
                for qc in range(4):
                    off = PO_OFF[qc]
                    inv = attnp.tile([128, 1], F32, tag="inv", bufs=4,
                                     name="inv")
                    nc.vector.reciprocal(inv, p_o[:, off + 128:off + 129])
                    o_sb = attnp.tile([128, HD], WDT, tag="osb", bufs=8,
                                      name="o_sb")
                    nc.vector.tensor_scalar_mul(o_sb, p_o[:, off:off + HD],
                                                inv)
                    p_tr = ps_tp2.tile([128, 128], WDT, tag="tp2", name="p_tr2")
                    nc.tensor.transpose(p_tr, o_sb, ident)
                    oT = resp.tile([128, 128], WDT, tag="oT", bufs=16,
                                   name="oT")
                    nc.scalar.copy(oT, p_tr)
                    oT_tiles[(h, qc)] = oT

            def out_proj_units(g, ps_py, oT_tiles, tail=False):
                for tt in range(4):
                    t = 4 * g + tt
                    for ec in range(4):
                        def u(t=t, tt=tt, ec=ec):
                            p_y = ps_py.tile([128, QT], F32, tag="py",
                                             name="p_y")
                            nc.tensor.matmul(
                                p_y, oT_tiles[(0, tt)],
                                wo_sb[:, 0, ec * 512:(ec + 1) * 512],
                                start=True, stop=False)
                            nc.tensor.matmul(
                                p_y, oT_tiles[(1, tt)],
                                wo_sb[:, 1, ec * 512:(ec + 1) * 512],
                                start=False, stop=True)
                            y = resp.tile([128, QT], F32, tag="y", bufs=4,
                                          name="y")
                            if tail and ec % 2 == 0:
                                nc.scalar.copy(y, p_y)
                            else:
                                nc.vector.tensor_copy(y, p_y)
                            eng = nc.gpsimd if ec % 2 == 0 else nc.sync
                            eng.dma_start(
                                out=out[t * LT:(t + 1) * LT,
                                        ec * 512:(ec + 1) * 512],
                                in_=y)
                        yield u

            with (
                tc.tile_pool(name="ps_qkv", bufs=2, space="PSUM") as ps_pv,
                tc.tile_pool(name="ps_tr", bufs=2, space="PSUM") as ps_tp1,
            ):
                # scalar HWDGE queue carries x tiles, interleaved with the
                # rope tables so neither gates the per-tile pipeline; x is
                # prefetched 4 tiles deep so the issue (which serializes
                # behind each tile's ACT chain) never gates the GEMM
                xs = {0: load_x(0)}
                nc.scalar.dma_start(out=w1_sb[:, 1:6, :], in_=w1[:, 1:6, :])
                nc.scalar.dma_start(out=w2_sb[:, 1:6, :], in_=w2[:, 1:6, :])
                xs[1] = load_x(1)
                xs[2] = load_x(2)
                nc.scalar.dma_start(out=w1_sb[:, 6:NT, :], in_=w1[:, 6:NT, :])
                nc.scalar.dma_start(out=w2_sb[:, 6:NT, :], in_=w2[:, 6:NT, :])
                xs[3] = load_x(3)
                xs[4] = load_x(4)
                phase1_tile(0, xs.pop(0), ps_pv, ps_tp1)
                for t in range(1, NT):
                    if t == 1:
                        nc.scalar.dma_start(out=wo_sb, in_=wo[:, :, :])
                    phase1_tile(t, xs.pop(t), ps_pv, ps_tp1)
                    if t + 4 < NT:
                        xs[t + 4] = load_x(t + 4)
            with (
                tc.tile_pool(name="ps_s", bufs=2, space="PSUM") as ps_qs,
                tc.tile_pool(name="ps_o", bufs=1, space="PSUM") as ps_po,
                tc.tile_pool(name="ps_y", bufs=2, space="PSUM") as ps_py,
                tc.tile_pool(name="ps_t2", bufs=2, space="PSUM") as ps_tp2,
            ):
                # group g's out-projection is spread one unit per chunk-step
                # through group g+1's attention to avoid head-of-line stalls
                units = iter(())
                for g in range(NQT):
                    oT_tiles = {}
                    attention_head(g, 0, ps_qs, ps_po, ps_tp2, oT_tiles, units)
                    attention_head(g, 1, ps_qs, ps_po, ps_tp2, oT_tiles, units)
                    for u in units:
                        u()
                    units = out_proj_units(g, ps_py, oT_tiles,
                                           tail=(g == NQT - 1))
                for u in units:
                    u()
    nc.compile()
    return nc


_NC_CACHE = None


def _get_nc():
    global _NC_CACHE
    if _NC_CACHE is None:
        _NC_CACHE = build()
    return _NC_CACHE


def prep_inputs(x, w_qkv, w_out):
    """Host-side sharding/layout prep. Returns list of per-core input maps."""
    wnp = ml_dtypes.bfloat16 if WDTYPE == "bf16" else np.float32
    x2d = np.asarray(x, dtype=np.float32).reshape(L, D)
    w_qkv = np.asarray(w_qkv, dtype=np.float32)
    w_out = np.asarray(w_out, dtype=np.float32)

    # xt[t, c, p, l] = x2d[t*128 + l, c*128 + p]
    # [t, p(hid), c, l] so each per-tile DMA is one linear stream
    xt = np.ascontiguousarray(
        x2d.reshape(NT, LT, NHC, HC).transpose(0, 3, 2, 1)).astype(wnp)

    # rope coefficient tables
    inv_freq = 1.0 / (ROPE_BASE ** (np.arange(0, HD, 2, dtype=np.float64) / HD))
    pos = np.arange(L, dtype=np.float64)[:, None]
    ang = pos * inv_freq[None, :]                       # [L, 64]
    cos, sin = np.cos(ang), np.sin(ang)
    w1 = np.zeros((L, HD), dtype=np.float32)
    w2 = np.zeros((L, HD), dtype=np.float32)
    w1[:, 0::2] = -sin
    w1[:, 1::2] = cos
    w2[:, 0::2] = cos
    w2[:, 1::2] = sin
    w1 = np.ascontiguousarray(w1.reshape(NT, LT, HD).transpose(1, 0, 2))
    w2 = np.ascontiguousarray(w2.reshape(NT, LT, HD).transpose(1, 0, 2))

    # causal mask tile for diagonal blocks
    i = np.arange(128)[:, None]
    j = np.arange(128)[None, :]
    mask4 = np.where(i <= j, 0.0, NEG).astype(np.float32)  # [128, 128]
    ident = np.eye(128, dtype=np.float32).astype(wnp)

    in_maps = []
    for c in range(N_CORES):
        h0 = HPC * c
        rows = []
        for part in range(3):  # q, k, v
            for hh in range(HPC):
                base = part * D + (h0 + hh) * HD
                rows.append(w_qkv[base:base + HD])
        w_c = np.concatenate(rows, axis=0)              # [768, D]
        wt = np.ascontiguousarray(w_c.T).astype(wnp)    # [D, 768]
        wo = np.ascontiguousarray(
            w_out[:, h0 * HD:(h0 + HPC) * HD].T.reshape(HPC, HD, D)
            .transpose(1, 0, 2)).astype(wnp)
        in_maps.append({
            "xt": xt, "wt": wt, "wo": wo, "w1": w1, "w2": w2,
            "mask4": mask4, "ident": ident,
        })
    return in_maps


def kernel(x, w_qkv, w_out, mask, _trace=False):
    """Full MHA forward. Returns [1, L, D] float32."""
    nc = _get_nc()
    in_maps = prep_inputs(x, w_qkv, w_out)
    res = run_bass_kernel_spmd(nc, in_maps, core_ids=list(range(N_CORES)),
                               trace=_trace)
    acc = np.zeros((L, D), dtype=np.float32)
    for r in res.results:
        acc += r["out"]
    out = acc.reshape(1, L, D)
    if _trace:
        return out, res
    return out
